# revision 9
# baseline (speedup 1.0000x reference)
"""Trainium2 Bass kernel for nn_BiquadCoeffFilter_31628139167986.

Reference computation (per batch row, T = 262144 samples):
  logits = linear-interp of 256 control points -> T samples (5 channels)
  a1 = 2*tanh(l0)*stab ; a2 = 0.5*((2-|a1|)*tanh(l1)*stab + |a1|)  (stab = 1-1e-3)
  IIR:  y[t] = x[t] - a1[t]*y[t-1] - a2[t]*y[t-2]
  FIR:  out[t] = b0[t]*y[t] + b1[t]*y[t-1] + b2[t]*y[t-2],  b = logits[..., 2:5]

Sharding: pure data parallel, 4 batch rows per core x 8 NeuronCores (SPMD).

Per-core pipeline (v2):
  A.  a-coefficient generation in SEGMENT-WINDOW layout (partition = one
      interpolation segment window of 1032 samples; the interpolated logit is
      affine in the in-window position, so the Activation engine computes
      tanh(w*d + v0) with per-partition scale/bias).  na1/na2 scattered to
      per-row time-linear DRAM stages, reloaded per row into the scan tile.
  A2. FIR b coefficients generated on the (otherwise idle) Activation engine
      in the same window layout, staged to DRAM; reloaded time-major into
      SBUF that the scan frees (x / a12 storage), overlapped under phase C.
  B.  Chunked 3-solution scan in SCAN layout [128 partitions = 8192-sample
      stretches, 256 chunks x 32 steps]: zero-state response y_zero +
      homogeneous h1 on DVE, h2 on GPSIMD.
  C.  Kogge-Stone prefix over the 256 per-chunk 2x2 affine transfer maps
      (flat [128,256] comps, row-1 DVE / row-2 GPSIMD), a [4,32] stretch-
      level KS via a tiny DRAM hop, per-chunk entry states alpha/beta, then
      the in-place correction y = y_zero + alpha*h1 + beta*h2.
  D.  FIR in time-major layout entirely on-chip (y stays in SBUF): 5 big
      shifted elementwise ops split DVE/GPSIMD + stretch-boundary fixups;
      one contiguous DMA writes the full output.
"""
import sys
sys.path.insert(0, '/opt/trn_rl_repo')
import numpy as np

B, T = 32, 262144
NSEG = 255
SEGLEN = 87381      # (T-1)/3 ; 3 super-blocks x 85 segments per row
SUP = 85
ROWS = 4
NCORES = 8
L1 = 32             # chunk length
NSTR = 32           # stretches per row
STR = T // NSTR     # 8192
CPS = STR // L1     # 256 chunks per stretch
WIN = 1032
PAD = 4
DELTA = float(NSEG) / float(T - 1)
STAB = 1.0 - 1e-3

_PATCHED = False


def _patch_tile_drain():
    """This toolchain allows a single sem wait per instruction; split the tile
    tail-drain's accumulated waits across chained drain instructions."""
    global _PATCHED
    if _PATCHED:
        return
    from concourse import tile, mybir
    from concourse.vector_clock import ScopedClock

    def _drain_and_barrier_split(self, tick_clock, wait_clock):
        drain_inst = self.nc.sync.drain()
        wait_clock.add_sem_waits(
            drain_inst.ins, ScopedClock({None: tick_clock.global_clock}))
        si = drain_inst.ins.sync_info
        waits = list(si.on_wait or []) if si else []
        if len(waits) > 1:
            si.on_wait = waits[:1]
            for i in range(1, len(waits)):
                d2 = self.nc.sync.drain()
                d2.ins.sync_info = mybir.SyncInfo(on_wait=[waits[i]], on_update=[])
        self.nc.all_engine_barrier()
        assert self.sems is not None
        popped = self.nc._tile_sem_poison_stack.pop()
        assert popped is self._sem_poison
        self.nc.clear_and_free_semaphores(list(self.sems.allocated().values()))
        self.nc.all_engine_barrier()

    tile.TileContext._drain_and_barrier = _drain_and_barrier_split
    _PATCHED = True


def _fix_multi_waits(nc):
    """Hoist extra sem waits onto same-engine nops (1-wait codegen limit)."""
    from concourse import mybir

    def make_nop(engine):
        bi = nc.engines[engine].nop(nofuse=True, hint="wait_split")
        inst = bi.ins
        for f in nc.m.functions:
            for bb in f.blocks:
                il = bb.instructions
                if il and il[-1] is inst:
                    bb.instructions = il[:-1]
                    return inst
        raise RuntimeError("nop not found")

    for f in nc.m.functions:
        for bb in f.blocks:
            il = list(bb.instructions)
            out = []
            changed = False
            for inst in il:
                si = getattr(inst, 'sync_info', None)
                waits = list(si.on_wait or []) if si else []
                if len(waits) > 1 and getattr(inst, 'engine', None) is not None:
                    changed = True
                    extra, keep = waits[:-1], waits[-1:]
                    for w in extra:
                        nop = make_nop(inst.engine)
                        nop.sync_info = mybir.SyncInfo(on_wait=[w], on_update=[])
                        out.append(nop)
                    si.on_wait = keep
                out.append(inst)
            if changed:
                bb.instructions = out
    return nc


def _lane_runs():
    """lane = r*255 + 85*k + sp  (row r, super-block k, segment sp).
    Runs of consecutive sp split at 128-partition tile boundaries.
    Returns (tile, part0, r, k, sp0, n)."""
    runs = []
    for r in range(ROWS):
        for k in range(3):
            base = r * NSEG + SUP * k
            sp = 0
            while sp < SUP:
                lane = base + sp
                tile_i, part = divmod(lane, 128)
                n = min(SUP - sp, 128 - part)
                runs.append((tile_i, part, r, k, sp, n))
                sp += n
    return runs


RUNS = _lane_runs()


def host_tables():
    w0 = np.zeros((128, 8, 1), np.float32)
    for r in range(ROWS):
        for k in range(3):
            for sp in range(SUP):
                lane = r * NSEG + SUP * k + sp
                seg = SUP * k + sp
                wstart = SEGLEN * k + 1028 * sp - 2
                w0[lane % 128, lane // 128, 0] = np.float64(wstart) * DELTA - seg
    iota = np.arange(WIN, dtype=np.float32)[None, :].repeat(128, 0)
    return w0, iota


def host_v0v1(cl_rows):
    """Per-lane control-point values [8,128,5] (pure data movement)."""
    v0 = np.zeros((128, 8, 5), np.float32)
    v1 = np.zeros((128, 8, 5), np.float32)
    for r in range(ROWS):
        for seg in range(NSEG):
            lane = r * NSEG + seg
            v0[lane % 128, lane // 128] = cl_rows[r, seg]
            v1[lane % 128, lane // 128] = cl_rows[r, seg + 1]
    return v0, v1


def build_program():
    from concourse import bass, mybir
    from concourse.tile import TileContext
    import bass_rust
    fp32 = mybir.dt.float32
    Alu = mybir.AluOpType
    Act = mybir.ActivationFunctionType

    nc = bass.Bass("TRN2", target_bir_lowering=False, debug=False)

    x_in = nc.dram_tensor("x", [ROWS, T], fp32, kind="ExternalInput").ap()
    v0_in = nc.dram_tensor("v0", [128, 8, 5], fp32, kind="ExternalInput").ap()
    v1_in = nc.dram_tensor("v1", [128, 8, 5], fp32, kind="ExternalInput").ap()
    w0_in = nc.dram_tensor("w0", [128, 8, 1], fp32, kind="ExternalInput").ap()
    iota_in = nc.dram_tensor("iota", [128, WIN], fp32, kind="ExternalInput").ap()
    y_out = nc.dram_tensor("y", [ROWS, T], fp32, kind="ExternalOutput").ap()

    st_nas = [nc.dram_tensor(f"st_na{r}", [2, T + 2 * PAD], fp32).ap()
              for r in range(ROWS)]
    st_b = nc.dram_tensor("st_b", [3, ROWS, T], fp32).ap()
    st_cmp = nc.dram_tensor("st_cmp", [128, 6], fp32).ap()
    st_sin = nc.dram_tensor("st_sin", [ROWS, NSTR, 2], fp32).ap()

    # DMA issuance on SP (idle -> no head-of-line blocking of compute
    # engines) except the A2 b-coefficient scatters (Activation issues its
    # own, overlapped under the scan).
    def dma(out, in_, eng=None):
        return (eng or nc.sync).dma_start(out=out, in_=in_)

    def stage_dst(base_ap, plane_stride, nplanes, start, n, ln):
        """[n segs (stride 1028), nplanes, ln cols] view into a DRAM stage."""
        v = base_ap.copy()
        v.ap = bass_rust.VecI64Pair([[1028, n], [plane_stride, nplanes],
                                     [1, ln]])
        v.offset = v.offset + start
        return v

    def scatter_planes(ti, tap, nplanes, mkbase, eng):
        """Scatter all planes of window tile [128, nplanes*WIN] to time-
        linear DRAM stages. mkbase(r) -> (base_ap, plane_stride, row_off)."""
        tv = tap.rearrange("p (c w) -> p c w", c=nplanes)
        for (tj, part, r, k, sp0, n) in [u for u in RUNS if u[0] == ti]:
            base_ap, ps, ro = mkbase(r)
            if sp0 == 0:
                dma(out=stage_dst(base_ap, ps, nplanes, ro + SEGLEN * k,
                                  1, 1029),
                    in_=tv[part:part + 1, :, 2:WIN - 1], eng=eng)
                if n > 1:
                    dma(out=stage_dst(base_ap, ps, nplanes,
                                      ro + SEGLEN * k + 1029, n - 1, 1028),
                        in_=tv[part + 1: part + n, :, 3:WIN - 1], eng=eng)
            else:
                dma(out=stage_dst(base_ap, ps, nplanes,
                                  ro + SEGLEN * k + 1028 * sp0 + 1, n, 1028),
                    in_=tv[part: part + n, :, 3:WIN - 1], eng=eng)
        for r in range(ROWS):
            lane = r * NSEG + NSEG - 1
            tj2, part2 = divmod(lane, 128)
            if tj2 == ti:
                base_ap, ps, ro = mkbase(r)
                dma(out=stage_dst(base_ap, ps, nplanes, ro + T - 1, 1, 1),
                    in_=tv[part2:part2 + 1, :, WIN - 1:WIN], eng=eng)

    na_base = [st_nas[r][0, :] for r in range(ROWS)]
    b_base = st_b[0, 0, :]

    with TileContext(nc) as tc:
      with tc.tile_pool(name="outer", bufs=1) as outer, \
           tc.tile_pool(name="consts", bufs=1) as consts:
        t_x = outer.tile([128, CPS, L1], fp32, name="xs")
        t_a12 = outer.tile([128, 2, CPS, L1], fp32, name="a12")
        t_yz = outer.tile([128, CPS, L1], fp32, name="yzs")
        dma(out=t_x[:].rearrange("p a b -> p (a b)"),
            in_=x_in.rearrange("r (p s) -> (r p) s", p=NSTR))

        t_iota = consts.tile([128, WIN], fp32, name="iota_t")
        t_v0a = consts.tile([128, 8, 5], fp32, name="v0all")
        t_v1a = consts.tile([128, 8, 5], fp32, name="v1all")
        t_w0a = consts.tile([128, 8, 1], fp32, name="w0all")
        t_da = consts.tile([128, 8, 5], fp32, name="dall")
        t_scb = consts.tile([128, 8, 3], fp32, name="scb")
        t_bib = consts.tile([128, 8, 3], fp32, name="bib")
        dma(out=t_iota[:], in_=iota_in)
        dma(out=t_v0a[:], in_=v0_in)
        dma(out=t_v1a[:], in_=v1_in)
        dma(out=t_w0a[:], in_=w0_in)
        nc.vector.tensor_tensor(t_da[:], t_v1a[:], t_v0a[:], Alu.subtract)
        # b_j = (d_j*DELTA)*iota + (v0_j + d_j*w0)  per lane
        nc.vector.tensor_scalar_mul(t_scb[:], t_da[:, :, 2:5], DELTA)
        nc.vector.tensor_tensor(
            t_bib[:], t_da[:, :, 2:5],
            t_w0a[:].broadcast_to([128, 8, 3]), Alu.mult)
        nc.vector.tensor_tensor(t_bib[:], t_bib[:], t_v0a[:, :, 2:5], Alu.add)

        # ---------------- phase A: a-coefficient generation ----------------
        with tc.tile_pool(name="segp", bufs=2) as sp_pool:
            for ti in range(8):
                t_v0 = t_v0a[:, ti, :]
                t_d = t_da[:, ti, :]
                t_w0 = t_w0a[:, ti, :]

                t_w = sp_pool.tile([128, WIN], fp32, name=f"w_{ti}", tag="w")
                nc.scalar.activation(t_w[:], t_iota[:], Act.Identity,
                                     bias=t_w0[:, 0:1], scale=DELTA)
                t_t1 = sp_pool.tile([128, WIN], fp32, name=f"t1_{ti}", tag="t1")
                t_t2 = sp_pool.tile([128, WIN], fp32, name=f"t2_{ti}", tag="t2")
                nc.scalar.activation(t_t1[:], t_w[:], Act.Tanh,
                                     bias=t_v0[:, 0:1], scale=t_d[:, 0:1])
                nc.scalar.activation(t_t2[:], t_w[:], Act.Tanh,
                                     bias=t_v0[:, 1:2], scale=t_d[:, 1:2])
                t_u = sp_pool.tile([128, WIN], fp32, name=f"u_{ti}", tag="u")
                nc.scalar.activation(t_u[:], t_t1[:], Act.Abs)
                t_na = sp_pool.tile([128, 2 * WIN], fp32, name=f"na_{ti}",
                                    tag="na")
                t_na1 = t_na[:, 0:WIN]
                t_na2 = t_na[:, WIN:2 * WIN]
                nc.vector.tensor_scalar_mul(t_na1, t_t1[:], -2.0 * STAB)
                t_st2 = sp_pool.tile([128, WIN], fp32, name=f"st2_{ti}",
                                     tag="st2")
                nc.vector.tensor_scalar_mul(t_st2[:], t_t2[:], STAB)
                t_vv = sp_pool.tile([128, WIN], fp32, name=f"vv_{ti}", tag="vv")
                nc.vector.tensor_scalar(t_vv[:], t_st2[:], -1.0, 1.0,
                                        Alu.mult, Alu.add)
                t_uv = sp_pool.tile([128, WIN], fp32, name=f"uv_{ti}", tag="uv")
                nc.gpsimd.tensor_tensor(t_uv[:], t_u[:], t_vv[:], Alu.mult)
                nc.vector.scalar_tensor_tensor(t_na2, t_uv[:], -STAB, t_st2[:],
                                               Alu.mult, Alu.subtract)
                scatter_planes(
                    ti, t_na[:], 2,
                    lambda r: (na_base[r], T + 2 * PAD, PAD), nc.sync)

        # ---------------- phase A2: b-coefficient windows -----------------
        with tc.tile_pool(name="bgen", bufs=2) as bp:
            for ti in range(8):
                t_b = bp.tile([128, 3 * WIN], fp32, name=f"bw_{ti}", tag="bw")
                bv = t_b[:].rearrange("p (c w) -> p c w", c=3)
                for j in range(3):
                    nc.scalar.activation(bv[:, j, :], t_iota[:], Act.Identity,
                                         bias=t_bib[:, ti, j:j + 1],
                                         scale=t_scb[:, ti, j:j + 1])
                scatter_planes(
                    ti, t_b[:], 3,
                    lambda r: (b_base, ROWS * T, r * T), nc.scalar)

        # per-row a12 reloads: RAW-dep only on that row's scatters
        a1 = t_a12[:, 0, :, :]
        a2 = t_a12[:, 1, :, :]
        for r in range(ROWS):
            dma(out=t_a12[r * NSTR:(r + 1) * NSTR, 0]
                .rearrange("p a b -> p (a b)"),
                in_=st_nas[r][0, PAD:PAD + T].rearrange("(p s) -> p s", p=NSTR))
            dma(out=t_a12[r * NSTR:(r + 1) * NSTR, 1]
                .rearrange("p a b -> p (a b)"),
                in_=st_nas[r][1, PAD:PAD + T].rearrange("(p s) -> p s", p=NSTR))

        # ---------------- phase B: chunked 3-solution scan ----------------
        with tc.tile_pool(name="scanp", bufs=1) as pool:
            t_h1 = pool.tile([128, CPS, L1], fp32, name="h1s")
            t_h2 = pool.tile([128, CPS, L1], fp32, name="h2s")
            t_tmp1 = pool.tile([128, CPS], fp32, name="sc_t1")
            t_tmp2 = pool.tile([128, CPS], fp32, name="sc_t2")
            t_g1 = pool.tile([128, CPS], fp32, name="gp_t1")
            t_g2 = pool.tile([128, CPS], fp32, name="gp_t2")
            t_al = pool.tile([128, CPS], fp32, name="alph")
            t_be = pool.tile([128, CPS], fp32, name="beta")

            # s = 0:  yz0 = x0 ; h1 col0 = a1_0 ; h2 col0 = a2_0
            nc.scalar.activation(t_yz[:, :, 0], t_x[:, :, 0], Act.Copy)
            nc.scalar.activation(t_h1[:, :, 0], a1[:, :, 0], Act.Copy)
            nc.scalar.activation(t_h2[:, :, 0], a2[:, :, 0], Act.Copy)
            # s = 1
            nc.vector.tensor_tensor(t_tmp1[:], a1[:, :, 1], t_yz[:, :, 0],
                                    Alu.mult)
            nc.vector.tensor_tensor(t_yz[:, :, 1], t_x[:, :, 1], t_tmp1[:],
                                    Alu.add)
            nc.vector.tensor_tensor(t_tmp2[:], a1[:, :, 1], t_h1[:, :, 0],
                                    Alu.mult)
            nc.vector.tensor_tensor(t_h1[:, :, 1], t_tmp2[:], a2[:, :, 1],
                                    Alu.add)
            nc.gpsimd.tensor_tensor(t_h2[:, :, 1], a1[:, :, 1], t_h2[:, :, 0],
                                    Alu.mult)
            # s = 2 .. L1-1
            for s in range(2, L1):
                a1s = a1[:, :, s]
                a2s = a2[:, :, s]
                nc.vector.tensor_tensor(t_tmp1[:], a1s, t_yz[:, :, s - 1],
                                        Alu.mult)
                nc.vector.tensor_tensor(t_tmp2[:], a2s, t_yz[:, :, s - 2],
                                        Alu.mult)
                nc.vector.tensor_tensor(t_tmp1[:], t_tmp1[:], t_tmp2[:],
                                        Alu.add)
                nc.vector.tensor_tensor(t_yz[:, :, s], t_x[:, :, s], t_tmp1[:],
                                        Alu.add)
                nc.vector.tensor_tensor(t_tmp1[:], a1s, t_h1[:, :, s - 1],
                                        Alu.mult)
                nc.vector.tensor_tensor(t_tmp2[:], a2s, t_h1[:, :, s - 2],
                                        Alu.mult)
                nc.vector.tensor_tensor(t_h1[:, :, s], t_tmp1[:], t_tmp2[:],
                                        Alu.add)
                nc.gpsimd.tensor_tensor(t_g1[:], a1s, t_h2[:, :, s - 1],
                                        Alu.mult)
                nc.gpsimd.tensor_tensor(t_g2[:], a2s, t_h2[:, :, s - 2],
                                        Alu.mult)
                nc.gpsimd.tensor_tensor(t_h2[:, :, s], t_g1[:], t_g2[:],
                                        Alu.add)

            # b0/b1 reloads into the a12 planes (dead after the scan);
            # overlaps phase C entirely.
            for r in range(ROWS):
                dma(out=t_a12[r * NSTR:(r + 1) * NSTR, 0]
                    .rearrange("p a b -> p (a b)"),
                    in_=st_b[0, r, :].rearrange("(p s) -> p s", p=NSTR))
                dma(out=t_a12[r * NSTR:(r + 1) * NSTR, 1]
                    .rearrange("p a b -> p (a b)"),
                    in_=st_b[1, r, :].rearrange("(p s) -> p s", p=NSTR))

            # ---------------- phase C: Kogge-Stone combine ----------------
            # comps order: m11, m12, m21, m22, q1, q2
            # KS ping-pong buffers carved out of t_x (dead after the scan)
            xf = t_x[:].rearrange("p a b -> p (a b)")
            ksA = [xf[:, i * CPS:(i + 1) * CPS] for i in range(6)]
            ksB = [xf[:, (6 + i) * CPS:(7 + i) * CPS] for i in range(6)]
            srcs = [t_h1[:, :, L1 - 1], t_h2[:, :, L1 - 1],
                    t_h1[:, :, L1 - 2], t_h2[:, :, L1 - 2],
                    t_yz[:, :, L1 - 1], t_yz[:, :, L1 - 2]]
            for i in range(6):
                nc.scalar.activation(ksA[i], srcs[i], Act.Copy)

            def ks_compose(dst, hi, lo, off, w, tmps):
                """dst[c] = hi[c] o lo[c-off] for the w composable entries;
                row1 (m11,m12,q1) on DVE, row2 on GPSIMD."""
                (d11, d12, d21, d22, dq1, dq2) = [d[:, off:off + w] for d in dst]
                (h11, h12, h21, h22, hq1, hq2) = [h[:, off:off + w] for h in hi]
                (l11, l12, l21, l22, lq1, lq2) = [l[:, 0:w] for l in lo]
                (tA, tB, tC, tD) = tmps
                # row 1 (DVE)
                nc.vector.tensor_tensor(tA, h11, l11, Alu.mult)
                nc.vector.tensor_tensor(tB, h12, l21, Alu.mult)
                nc.vector.tensor_tensor(d11, tA, tB, Alu.add)
                nc.vector.tensor_tensor(tA, h11, l12, Alu.mult)
                nc.vector.tensor_tensor(tB, h12, l22, Alu.mult)
                nc.vector.tensor_tensor(d12, tA, tB, Alu.add)
                nc.vector.tensor_tensor(tA, h11, lq1, Alu.mult)
                nc.vector.tensor_tensor(tB, h12, lq2, Alu.mult)
                nc.vector.tensor_tensor(tA, tA, tB, Alu.add)
                nc.vector.tensor_tensor(dq1, tA, hq1, Alu.add)
                # row 2 (GPSIMD)
                nc.gpsimd.tensor_tensor(tC, h21, l11, Alu.mult)
                nc.gpsimd.tensor_tensor(tD, h22, l21, Alu.mult)
                nc.gpsimd.tensor_tensor(d21, tC, tD, Alu.add)
                nc.gpsimd.tensor_tensor(tC, h21, l12, Alu.mult)
                nc.gpsimd.tensor_tensor(tD, h22, l22, Alu.mult)
                nc.gpsimd.tensor_tensor(d22, tC, tD, Alu.add)
                nc.gpsimd.tensor_tensor(tC, h21, lq1, Alu.mult)
                nc.gpsimd.tensor_tensor(tD, h22, lq2, Alu.mult)
                nc.gpsimd.tensor_tensor(tC, tC, tD, Alu.add)
                nc.gpsimd.tensor_tensor(dq2, tC, hq2, Alu.add)

            cur, nxt = ksA, ksB
            off = 1
            while off < CPS:
                for i in range(6):  # pass-through prefix entries
                    nc.scalar.activation(nxt[i][:, 0:off], cur[i][:, 0:off],
                                         Act.Copy)
                w = CPS - off
                ks_compose(nxt, cur, cur, off, w,
                           (t_tmp1[:, 0:w], t_tmp2[:, 0:w],
                            t_g1[:, 0:w], t_g2[:, 0:w]))
                cur, nxt = nxt, cur
                off *= 2

            # stretch composites -> DRAM (st_cmp[p, c] = comp c of partition p)
            t_cmp = pool.tile([128, 6], fp32, name="cmp")
            for i in range(6):
                nc.scalar.activation(t_cmp[:, i:i + 1], cur[i][:, CPS - 1:CPS],
                                     Act.Copy)
            dma(out=st_cmp, in_=t_cmp[:])

            # row-level KS over the 32 stretches of each row (4 partitions)
            t_row = pool.tile([4, NSTR, 6], fp32, name="rowc")
            dma(out=t_row[:], in_=st_cmp.rearrange("(r j) c -> r j c", r=ROWS))
            rA = [pool.tile([4, NSTR], fp32, name=f"rA{i}") for i in range(6)]
            rB = [pool.tile([4, NSTR], fp32, name=f"rB{i}") for i in range(6)]
            rt = [pool.tile([4, NSTR], fp32, name=f"rt{i}") for i in range(4)]
            for i in range(6):
                nc.scalar.activation(rA[i][:], t_row[:, :, i], Act.Copy)
            rcur, rnxt = rA, rB
            off = 1
            while off < NSTR:
                for i in range(6):
                    nc.scalar.activation(rnxt[i][:, 0:off], rcur[i][:, 0:off],
                                         Act.Copy)
                w = NSTR - off
                ks_compose([rr[:] for rr in rnxt], [rr[:] for rr in rcur],
                           [rr[:] for rr in rcur], off, w,
                           tuple(rr[:, 0:w] for rr in (rt[0][:], rt[1][:],
                                                       rt[2][:], rt[3][:])))
                rcur, rnxt = rnxt, rcur
                off *= 2

            # stretch entry states: s_entry[j] = (q1, q2) of prefix[j-1]
            sent = pool.tile([4, NSTR, 2], fp32, name="sent")
            nc.vector.memset(sent[:, 0, :], 0.0)
            nc.vector.tensor_copy(sent[:, 1:NSTR, 0], rcur[4][:, 0:NSTR - 1])
            nc.vector.tensor_copy(sent[:, 1:NSTR, 1], rcur[5][:, 0:NSTR - 1])
            dma(out=st_sin, in_=sent[:])

            # back to scan layout: per-partition stretch entry [128, 2]
            t_sstr = pool.tile([128, 2], fp32, name="sstr")
            dma(out=t_sstr[:], in_=st_sin.rearrange("r j c -> (r j) c"))

            # chunk entry states: alpha/beta [128, CPS]
            # alpha[0] = s1 ; alpha[c] = m11p[c-1] s1 + m12p[c-1] s2 + q1p[c-1]
            s1 = t_sstr[:, 0:1]
            s2 = t_sstr[:, 1:2]
            nc.vector.tensor_copy(t_al[:, 0:1], s1)
            nc.vector.tensor_copy(t_be[:, 0:1], s2)
            W = CPS - 1
            # TensorScalarPtr (AP-scalar) ops are DVE-only; plain adds on GP
            nc.vector.tensor_scalar_mul(t_tmp1[:, 0:W], cur[0][:, 0:W], s1)
            nc.vector.scalar_tensor_tensor(t_al[:, 1:CPS], cur[1][:, 0:W], s2,
                                           t_tmp1[:, 0:W], Alu.mult, Alu.add)
            nc.gpsimd.tensor_tensor(t_al[:, 1:CPS], t_al[:, 1:CPS],
                                    cur[4][:, 0:W], Alu.add)
            nc.vector.tensor_scalar_mul(t_tmp2[:, 0:W], cur[2][:, 0:W], s1)
            nc.vector.scalar_tensor_tensor(t_be[:, 1:CPS], cur[3][:, 0:W], s2,
                                           t_tmp2[:, 0:W], Alu.mult, Alu.add)
            nc.gpsimd.tensor_tensor(t_be[:, 1:CPS], t_be[:, 1:CPS],
                                    cur[5][:, 0:W], Alu.add)

            # b2 reload into t_x (KS buffers dead after alpha/beta)
            for r in range(ROWS):
                dma(out=t_x[r * NSTR:(r + 1) * NSTR]
                    .rearrange("p a b -> p (a b)"),
                    in_=st_b[2, r, :].rearrange("(p s) -> p s", p=NSTR))

            # ---- correction (in place): y = y_zero + alpha*h1 + beta*h2 ----
            alb = t_al[:].unsqueeze(-1).broadcast_to([128, CPS, L1])
            beb = t_be[:].unsqueeze(-1).broadcast_to([128, CPS, L1])
            C0 = 186  # DVE/GPSIMD column split (~73/27)
            nc.vector.tensor_tensor(t_h1[:, 0:C0], t_h1[:, 0:C0], alb[:, 0:C0],
                                    Alu.mult)
            nc.vector.tensor_tensor(t_yz[:, 0:C0], t_yz[:, 0:C0],
                                    t_h1[:, 0:C0], Alu.add)
            nc.vector.tensor_tensor(t_h2[:, 0:C0], t_h2[:, 0:C0], beb[:, 0:C0],
                                    Alu.mult)
            nc.vector.tensor_tensor(t_yz[:, 0:C0], t_yz[:, 0:C0],
                                    t_h2[:, 0:C0], Alu.add)
            nc.gpsimd.tensor_tensor(t_h1[:, C0:CPS], t_h1[:, C0:CPS],
                                    alb[:, C0:CPS], Alu.mult)
            nc.gpsimd.tensor_tensor(t_yz[:, C0:CPS], t_yz[:, C0:CPS],
                                    t_h1[:, C0:CPS], Alu.add)
            nc.gpsimd.tensor_tensor(t_h2[:, C0:CPS], t_h2[:, C0:CPS],
                                    beb[:, C0:CPS], Alu.mult)
            nc.gpsimd.tensor_tensor(t_yz[:, C0:CPS], t_yz[:, C0:CPS],
                                    t_h2[:, C0:CPS], Alu.add)

            # ------------- phase D: FIR, time-major, on-chip -------------
            yzf = t_yz[:].rearrange("p a b -> p (a b)")
            resf = t_h1[:].rearrange("p a b -> p (a b)")   # result
            tmpf = t_h2[:].rearrange("p a b -> p (a b)")   # scratch
            b0f = t_a12[:, 0].rearrange("p a b -> p (a b)")
            b1f = t_a12[:, 1].rearrange("p a b -> p (a b)")
            b2f = xf

            # stretch-boundary history: t_ysh[p] = (y[-2], y[-1]) of stretch p
            t_ysh = pool.tile([128, 2], fp32, name="ysh")
            nc.vector.memset(t_ysh[:], 0.0)
            for r in range(ROWS):
                dma(out=t_ysh[r * NSTR + 1:(r + 1) * NSTR, :],
                    in_=yzf[r * NSTR:(r + 1) * NSTR - 1, STR - 2:STR])

            K = 5547  # DVE/GPSIMD column split for the big FIR ops
            E = STR
            # res = b0 * y
            nc.vector.tensor_tensor(resf[:, 0:K], b0f[:, 0:K], yzf[:, 0:K],
                                    Alu.mult)
            nc.gpsimd.tensor_tensor(resf[:, K:E], b0f[:, K:E], yzf[:, K:E],
                                    Alu.mult)
            # res[t] += b1[t] * y[t-1]
            nc.vector.tensor_tensor(tmpf[:, 1:K], b1f[:, 1:K], yzf[:, 0:K - 1],
                                    Alu.mult)
            nc.gpsimd.tensor_tensor(tmpf[:, K:E], b1f[:, K:E],
                                    yzf[:, K - 1:E - 1], Alu.mult)
            nc.vector.tensor_tensor(resf[:, 1:K], resf[:, 1:K], tmpf[:, 1:K],
                                    Alu.add)
            nc.gpsimd.tensor_tensor(resf[:, K:E], resf[:, K:E], tmpf[:, K:E],
                                    Alu.add)
            # res[t] += b2[t] * y[t-2]
            nc.vector.tensor_tensor(tmpf[:, 2:K], b2f[:, 2:K], yzf[:, 0:K - 2],
                                    Alu.mult)
            nc.gpsimd.tensor_tensor(tmpf[:, K:E], b2f[:, K:E],
                                    yzf[:, K - 2:E - 2], Alu.mult)
            nc.vector.tensor_tensor(resf[:, 2:K], resf[:, 2:K], tmpf[:, 2:K],
                                    Alu.add)
            nc.gpsimd.tensor_tensor(resf[:, K:E], resf[:, K:E], tmpf[:, K:E],
                                    Alu.add)
            # stretch-boundary fixups for cols 0..1 (history from t_ysh)
            fx = t_tmp1[:, 0:1]
            nc.vector.tensor_tensor(fx, b1f[:, 0:1], t_ysh[:, 1:2], Alu.mult)
            nc.vector.tensor_tensor(resf[:, 0:1], resf[:, 0:1], fx, Alu.add)
            nc.vector.tensor_tensor(fx, b2f[:, 0:1], t_ysh[:, 0:1], Alu.mult)
            nc.vector.tensor_tensor(resf[:, 0:1], resf[:, 0:1], fx, Alu.add)
            nc.vector.tensor_tensor(fx, b2f[:, 1:2], t_ysh[:, 1:2], Alu.mult)
            nc.vector.tensor_tensor(resf[:, 1:2], resf[:, 1:2], fx, Alu.add)

            # output: one contiguous DMA
            dma(out=y_out.rearrange("r (p s) -> (r p) s", p=NSTR), in_=resf)

    _fix_multi_waits(nc)
    return nc


_NC_CACHE = None
LAST_EXEC_NS = None


def _register_ntff_hook():
    """Make antenv.axon_hooks importable and register the ctypes NTFF hook so
    run_bass_kernel_spmd(trace=True) can measure real device exec time."""
    import types
    name = 'antenv.axon_hooks'
    if name not in sys.modules:
        mod = types.ModuleType(name)
        holder = [None]
        mod.set_axon_ntff_profile_hook = lambda h: holder.__setitem__(0, h)
        mod.get_axon_ntff_profile_hook = lambda: holder[0]
        import antenv
        antenv.axon_hooks = mod
        sys.modules[name] = mod
    if sys.modules[name].get_axon_ntff_profile_hook() is None:
        from trn_agent_boot.trn_boot import _ntff_profile_via_ctypes
        hook = _ntff_profile_via_ctypes('/opt/axon/libaxon_pjrt.so')
        sys.modules[name].set_axon_ntff_profile_hook(hook)


def kernel(x, coeff_logits):
    """Full inputs -> full output, running the Bass kernel on 8 NeuronCores."""
    global _NC_CACHE, LAST_EXEC_NS
    _patch_tile_drain()
    from concourse.bass_utils import run_bass_kernel_spmd

    x = np.ascontiguousarray(np.asarray(x, dtype=np.float32))
    cl = np.ascontiguousarray(np.asarray(coeff_logits, dtype=np.float32))
    if _NC_CACHE is None:
        _NC_CACHE = build_program()
    nc = _NC_CACHE

    w0, iota = host_tables()
    in_maps = []
    for c in range(NCORES):
        rows = slice(c * ROWS, (c + 1) * ROWS)
        v0, v1 = host_v0v1(cl[rows])
        in_maps.append({
            "x": x[rows].copy(),
            "v0": v0, "v1": v1, "w0": w0, "iota": iota,
        })
    import os
    mode = os.environ.get("KERNEL_TIME", "0")
    if mode == "ntff":
        # real device timing + perfetto trace via the NTFF profile hook
        _register_ntff_hook()
        from concourse import bass_utils as _bu
        _bu.upload_artifacts = lambda tmpdir: "local://" + tmpdir
        outdir = os.environ.get("PROF_OUT", "/tmp/kernel_prof")
        os.makedirs(outdir, exist_ok=True)
        res = run_bass_kernel_spmd(nc, in_maps, list(range(NCORES)),
                                   tmpdir=outdir, trace=True, trace_cores=[0])
        LAST_EXEC_NS = res.exec_time_ns
    elif mode == "1":
        import time, jax
        cap = {}
        orig_jit = jax.jit

        def capturing_jit(f, **kw):
            j = orig_jit(f, **kw)

            def wrapper(*a, **k):
                cap['fn'], cap['args'] = j, a
                return j(*a, **k)
            return wrapper

        jax.jit = capturing_jit
        try:
            res = run_bass_kernel_spmd(nc, in_maps, list(range(NCORES)))
        finally:
            jax.jit = orig_jit
        try:
            fn, args = cap['fn'], cap['args']
            jax.block_until_ready(fn(*args))  # warm
            K = 5
            t0 = time.perf_counter()
            for _ in range(K):
                o = fn(*args)
            jax.block_until_ready(o)
            LAST_EXEC_NS = int((time.perf_counter() - t0) / K * 1e9)
        except Exception as e:
            print("timing failed:", e)
            LAST_EXEC_NS = -1
    else:
        res = run_bass_kernel_spmd(nc, in_maps, list(range(NCORES)))
    out = np.empty((B, T), np.float32)
    for c in range(NCORES):
        out[c * ROWS:(c + 1) * ROWS] = res.results[c]["y"]
    return out


# revision 10
# speedup vs baseline: 1.1869x; 1.1869x over previous
"""Trainium2 Bass kernel for nn_BiquadCoeffFilter_31628139167986.

Reference computation (per batch row, T = 262144 samples):
  logits = linear-interp of 256 control points -> T samples (5 channels)
  a1 = 2*tanh(l0)*stab ; a2 = 0.5*((2-|a1|)*tanh(l1)*stab + |a1|)  (stab = 1-1e-3)
  IIR:  y[t] = x[t] - a1[t]*y[t-1] - a2[t]*y[t-2]
  FIR:  out[t] = b0[t]*y[t] + b1[t]*y[t-1] + b2[t]*y[t-2],  b = logits[..., 2:5]

Sharding: pure data parallel, 4 batch rows per core x 8 NeuronCores (SPMD).

Per-core pipeline (v3):
  A. a-coefficient generation in SEGMENT-WINDOW layout (partition = one
     interpolation segment window of 1032 samples; the interpolated logit is
     affine in the in-window position, so the Activation engine computes
     tanh(w*d + v0) with per-partition scale/bias).  na1/na2 scattered to
     per-row time-linear DRAM stages (issues split SP/GPSIMD), reloaded per
     row into the scan tile.
  B. Chunked 3-solution scan in SCAN layout [128 partitions = 8192-sample
     stretches, 256 chunks x 32 steps]: zero-state response y_zero +
     homogeneous h1 on DVE, h2 on GPSIMD.
  C. Kogge-Stone prefix over the 256 per-chunk 2x2 affine transfer maps
     (flat [128,256] comps; 13 ops on DVE / 7 on GPSIMD per level), a [4,32]
     stretch-level KS via a tiny DRAM hop, per-chunk entry states
     alpha/beta, then the in-place correction y += alpha*h1 + beta*h2.
  D. FIR in segment-window layout: corrected y staged to DRAM per row,
     windows gathered back (deep prefetch), b coefficients generated on the
     Activation engine, output scattered straight to DRAM (issues split
     SP/GPSIMD).
"""
import sys
sys.path.insert(0, '/opt/trn_rl_repo')
import numpy as np

B, T = 32, 262144
NSEG = 255
SEGLEN = 87381      # (T-1)/3 ; 3 super-blocks x 85 segments per row
SUP = 85
ROWS = 4
NCORES = 8
L1 = 32             # chunk length
NSTR = 32           # stretches per row
STR = T // NSTR     # 8192
CPS = STR // L1     # 256 chunks per stretch
WIN = 1032
PAD = 4
DELTA = float(NSEG) / float(T - 1)
STAB = 1.0 - 1e-3

_PATCHED = False


def _patch_tile_drain():
    """This toolchain allows a single sem wait per instruction; split the tile
    tail-drain's accumulated waits across chained drain instructions."""
    global _PATCHED
    if _PATCHED:
        return
    from concourse import tile, mybir
    from concourse.vector_clock import ScopedClock

    def _drain_and_barrier_split(self, tick_clock, wait_clock):
        drain_inst = self.nc.sync.drain()
        wait_clock.add_sem_waits(
            drain_inst.ins, ScopedClock({None: tick_clock.global_clock}))
        si = drain_inst.ins.sync_info
        waits = list(si.on_wait or []) if si else []
        if len(waits) > 1:
            si.on_wait = waits[:1]
            for i in range(1, len(waits)):
                d2 = self.nc.sync.drain()
                d2.ins.sync_info = mybir.SyncInfo(on_wait=[waits[i]], on_update=[])
        self.nc.all_engine_barrier()
        assert self.sems is not None
        popped = self.nc._tile_sem_poison_stack.pop()
        assert popped is self._sem_poison
        self.nc.clear_and_free_semaphores(list(self.sems.allocated().values()))
        self.nc.all_engine_barrier()

    tile.TileContext._drain_and_barrier = _drain_and_barrier_split
    _PATCHED = True


def _fix_multi_waits(nc):
    """Hoist extra sem waits onto same-engine nops (1-wait codegen limit)."""
    from concourse import mybir

    def make_nop(engine):
        bi = nc.engines[engine].nop(nofuse=True, hint="wait_split")
        inst = bi.ins
        for f in nc.m.functions:
            for bb in f.blocks:
                il = bb.instructions
                if il and il[-1] is inst:
                    bb.instructions = il[:-1]
                    return inst
        raise RuntimeError("nop not found")

    for f in nc.m.functions:
        for bb in f.blocks:
            il = list(bb.instructions)
            out = []
            changed = False
            for inst in il:
                si = getattr(inst, 'sync_info', None)
                waits = list(si.on_wait or []) if si else []
                if len(waits) > 1 and getattr(inst, 'engine', None) is not None:
                    changed = True
                    extra, keep = waits[:-1], waits[-1:]
                    for w in extra:
                        nop = make_nop(inst.engine)
                        nop.sync_info = mybir.SyncInfo(on_wait=[w], on_update=[])
                        out.append(nop)
                    si.on_wait = keep
                out.append(inst)
            if changed:
                bb.instructions = out
    return nc


def _lane_runs():
    """lane = r*255 + 85*k + sp  (row r, super-block k, segment sp).
    Runs of consecutive sp split at 128-partition tile boundaries.
    Returns (tile, part0, r, k, sp0, n)."""
    runs = []
    for r in range(ROWS):
        for k in range(3):
            base = r * NSEG + SUP * k
            sp = 0
            while sp < SUP:
                lane = base + sp
                tile_i, part = divmod(lane, 128)
                n = min(SUP - sp, 128 - part)
                runs.append((tile_i, part, r, k, sp, n))
                sp += n
    return runs


RUNS = _lane_runs()


def host_tables():
    w0 = np.zeros((128, 8, 1), np.float32)
    for r in range(ROWS):
        for k in range(3):
            for sp in range(SUP):
                lane = r * NSEG + SUP * k + sp
                seg = SUP * k + sp
                wstart = SEGLEN * k + 1028 * sp - 2
                w0[lane % 128, lane // 128, 0] = np.float64(wstart) * DELTA - seg
    iota = np.arange(WIN, dtype=np.float32)[None, :].repeat(128, 0)
    return w0, iota


def host_v0v1(cl_rows):
    """Per-lane control-point values [8,128,5] (pure data movement)."""
    v0 = np.zeros((128, 8, 5), np.float32)
    v1 = np.zeros((128, 8, 5), np.float32)
    for r in range(ROWS):
        for seg in range(NSEG):
            lane = r * NSEG + seg
            v0[lane % 128, lane // 128] = cl_rows[r, seg]
            v1[lane % 128, lane // 128] = cl_rows[r, seg + 1]
    return v0, v1


def build_program():
    from concourse import bass, mybir
    from concourse.tile import TileContext
    import bass_rust
    fp32 = mybir.dt.float32
    Alu = mybir.AluOpType
    Act = mybir.ActivationFunctionType

    nc = bass.Bass("TRN2", target_bir_lowering=False, debug=False)

    x_in = nc.dram_tensor("x", [ROWS, T], fp32, kind="ExternalInput").ap()
    v0_in = nc.dram_tensor("v0", [128, 8, 5], fp32, kind="ExternalInput").ap()
    v1_in = nc.dram_tensor("v1", [128, 8, 5], fp32, kind="ExternalInput").ap()
    w0_in = nc.dram_tensor("w0", [128, 8, 1], fp32, kind="ExternalInput").ap()
    iota_in = nc.dram_tensor("iota", [128, WIN], fp32, kind="ExternalInput").ap()
    y_out = nc.dram_tensor("y", [ROWS, T], fp32, kind="ExternalOutput").ap()

    st_nas = [nc.dram_tensor(f"st_na{r}", [2, T + 2 * PAD], fp32).ap()
              for r in range(ROWS)]
    st_y = nc.dram_tensor("st_y", [ROWS, T + 2 * PAD], fp32).ap()
    st_cmp = nc.dram_tensor("st_cmp", [128, 6], fp32).ap()
    st_sin = nc.dram_tensor("st_sin", [ROWS, NSTR, 2], fp32).ap()

    # Scatter-heavy DMA issuance alternates SP / GPSIMD (both near idle at
    # scatter time); everything latency-critical goes on SP.
    _sc_engines = [nc.sync, nc.gpsimd]
    _sc_rr = [0]

    def dma(out, in_, eng=None):
        return (eng or nc.sync).dma_start(out=out, in_=in_)

    def dma_rr(out, in_):
        eng = _sc_engines[_sc_rr[0] % len(_sc_engines)]
        _sc_rr[0] += 1
        return eng.dma_start(out=out, in_=in_)

    def win_src(st, r, start, n):
        v = st[r, :].copy()
        v.ap = bass_rust.VecI64Pair([[1028, n], [1, WIN]])
        v.offset = v.offset + start
        return v

    def na_dst(r, start, n, ln):
        # [n segs (stride 1028), 2 planes, ln cols] view into row-r na stage
        v = st_nas[r][0, :].copy()
        v.ap = bass_rust.VecI64Pair([[1028, n], [T + 2 * PAD, 2], [1, ln]])
        v.offset = v.offset + start
        return v

    def scatter_na(ti, tap):
        """Scatter BOTH na planes of the paired tile [128, 2*WIN] at once."""
        tv = tap.rearrange("p (c w) -> p c w", c=2)
        for (tj, part, r, k, sp0, n) in [u for u in RUNS if u[0] == ti]:
            if sp0 == 0:
                dma_rr(out=na_dst(r, PAD + SEGLEN * k, 1, 1029),
                       in_=tv[part:part + 1, :, 2:WIN - 1])
                if n > 1:
                    dma_rr(out=na_dst(r, PAD + SEGLEN * k + 1029, n - 1, 1028),
                           in_=tv[part + 1: part + n, :, 3:WIN - 1])
            else:
                dma_rr(out=na_dst(r, PAD + SEGLEN * k + 1028 * sp0 + 1, n, 1028),
                       in_=tv[part: part + n, :, 3:WIN - 1])
        for r in range(ROWS):
            lane = r * NSEG + NSEG - 1
            tj2, part2 = divmod(lane, 128)
            if tj2 == ti:
                dma_rr(out=na_dst(r, PAD + T - 1, 1, 1),
                       in_=tv[part2:part2 + 1, :, WIN - 1:WIN])

    def scatter_tile(ti, tap, st, off):
        """Write true-segment cols of window tile `ti` to time-linear layout."""
        for (tj, part, r, k, sp0, n) in [u for u in RUNS if u[0] == ti]:
            if sp0 == 0:
                base_t = SEGLEN * k
                dma_rr(out=st[r:r + 1, off + base_t: off + base_t + 1029],
                       in_=tap[part:part + 1, 2:WIN - 1])
                if n > 1:
                    base_t = SEGLEN * k + 1029
                    dma_rr(
                        out=st[r, off + base_t: off + base_t + (n - 1) * 1028]
                            .rearrange("(a b) -> a b", b=1028),
                        in_=tap[part + 1: part + n, 3:WIN - 1])
            else:
                base_t = SEGLEN * k + 1028 * sp0 + 1
                dma_rr(
                    out=st[r, off + base_t: off + base_t + n * 1028]
                        .rearrange("(a b) -> a b", b=1028),
                    in_=tap[part: part + n, 3:WIN - 1])
        # last sample t = T-1 comes from the final segment's window col 1031
        for r in range(ROWS):
            lane = r * NSEG + NSEG - 1
            tj2, part2 = divmod(lane, 128)
            if tj2 == ti:
                dma_rr(out=st[r:r + 1, off + T - 1: off + T],
                       in_=tap[part2:part2 + 1, WIN - 1:WIN])

    with TileContext(nc) as tc:
      with tc.tile_pool(name="outer", bufs=1) as outer, \
           tc.tile_pool(name="consts", bufs=1) as consts:
        t_x = outer.tile([128, CPS, L1], fp32, name="xs")
        t_a12 = outer.tile([128, 2, CPS, L1], fp32, name="a12")
        t_yz = outer.tile([128, CPS, L1], fp32, name="yzs")
        dma(out=t_x[:].rearrange("p a b -> p (a b)"),
            in_=x_in.rearrange("r (p s) -> (r p) s", p=NSTR))

        t_iota = consts.tile([128, WIN], fp32, name="iota_t")
        t_v0a = consts.tile([128, 8, 5], fp32, name="v0all")
        t_v1a = consts.tile([128, 8, 5], fp32, name="v1all")
        t_w0a = consts.tile([128, 8, 1], fp32, name="w0all")
        t_da = consts.tile([128, 8, 5], fp32, name="dall")
        dma(out=t_iota[:], in_=iota_in)
        dma(out=t_v0a[:], in_=v0_in)
        dma(out=t_v1a[:], in_=v1_in)
        dma(out=t_w0a[:], in_=w0_in)
        nc.vector.tensor_tensor(t_da[:], t_v1a[:], t_v0a[:], Alu.subtract)

        # ---------------- phase A: a-coefficient generation ----------------
        with tc.tile_pool(name="segp", bufs=2) as sp_pool:
            for ti in range(8):
                t_v0 = t_v0a[:, ti, :]
                t_d = t_da[:, ti, :]
                t_w0 = t_w0a[:, ti, :]

                t_w = sp_pool.tile([128, WIN], fp32, name=f"w_{ti}", tag="w")
                nc.scalar.activation(t_w[:], t_iota[:], Act.Identity,
                                     bias=t_w0[:, 0:1], scale=DELTA)
                t_t1 = sp_pool.tile([128, WIN], fp32, name=f"t1_{ti}", tag="t1")
                t_t2 = sp_pool.tile([128, WIN], fp32, name=f"t2_{ti}", tag="t2")
                nc.scalar.activation(t_t1[:], t_w[:], Act.Tanh,
                                     bias=t_v0[:, 0:1], scale=t_d[:, 0:1])
                nc.scalar.activation(t_t2[:], t_w[:], Act.Tanh,
                                     bias=t_v0[:, 1:2], scale=t_d[:, 1:2])
                t_u = sp_pool.tile([128, WIN], fp32, name=f"u_{ti}", tag="u")
                nc.scalar.activation(t_u[:], t_t1[:], Act.Abs)
                t_na = sp_pool.tile([128, 2 * WIN], fp32, name=f"na_{ti}",
                                    tag="na")
                t_na1 = t_na[:, 0:WIN]
                t_na2 = t_na[:, WIN:2 * WIN]
                nc.vector.tensor_scalar_mul(t_na1, t_t1[:], -2.0 * STAB)
                t_st2 = sp_pool.tile([128, WIN], fp32, name=f"st2_{ti}",
                                     tag="st2")
                nc.vector.tensor_scalar_mul(t_st2[:], t_t2[:], STAB)
                t_vv = sp_pool.tile([128, WIN], fp32, name=f"vv_{ti}", tag="vv")
                nc.vector.tensor_scalar(t_vv[:], t_st2[:], -1.0, 1.0,
                                        Alu.mult, Alu.add)
                t_uv = sp_pool.tile([128, WIN], fp32, name=f"uv_{ti}", tag="uv")
                nc.gpsimd.tensor_tensor(t_uv[:], t_u[:], t_vv[:], Alu.mult)
                nc.vector.scalar_tensor_tensor(t_na2, t_uv[:], -STAB, t_st2[:],
                                               Alu.mult, Alu.subtract)
                scatter_na(ti, t_na[:])

        # per-row a12 reloads: RAW-dep only on that row's scatters
        a1 = t_a12[:, 0, :, :]
        a2 = t_a12[:, 1, :, :]
        for r in range(ROWS):
            dma(out=t_a12[r * NSTR:(r + 1) * NSTR, 0]
                .rearrange("p a b -> p (a b)"),
                in_=st_nas[r][0, PAD:PAD + T].rearrange("(p s) -> p s", p=NSTR))
            dma(out=t_a12[r * NSTR:(r + 1) * NSTR, 1]
                .rearrange("p a b -> p (a b)"),
                in_=st_nas[r][1, PAD:PAD + T].rearrange("(p s) -> p s", p=NSTR))

        # ---------------- phase B: chunked 3-solution scan ----------------
        with tc.tile_pool(name="scanp", bufs=1) as pool:
            t_h1 = pool.tile([128, CPS, L1], fp32, name="h1s")
            t_h2 = pool.tile([128, CPS, L1], fp32, name="h2s")
            t_tmp1 = pool.tile([128, CPS], fp32, name="sc_t1")
            t_tmp2 = pool.tile([128, CPS], fp32, name="sc_t2")
            t_g1 = pool.tile([128, CPS], fp32, name="gp_t1")
            t_g2 = pool.tile([128, CPS], fp32, name="gp_t2")
            t_al = pool.tile([128, CPS], fp32, name="alph")
            t_be = pool.tile([128, CPS], fp32, name="beta")

            # s = 0:  yz0 = x0 ; h1 col0 = a1_0 ; h2 col0 = a2_0
            nc.scalar.activation(t_yz[:, :, 0], t_x[:, :, 0], Act.Copy)
            nc.scalar.activation(t_h1[:, :, 0], a1[:, :, 0], Act.Copy)
            nc.scalar.activation(t_h2[:, :, 0], a2[:, :, 0], Act.Copy)
            # s = 1
            nc.vector.tensor_tensor(t_tmp1[:], a1[:, :, 1], t_yz[:, :, 0],
                                    Alu.mult)
            nc.vector.tensor_tensor(t_yz[:, :, 1], t_x[:, :, 1], t_tmp1[:],
                                    Alu.add)
            nc.vector.tensor_tensor(t_tmp2[:], a1[:, :, 1], t_h1[:, :, 0],
                                    Alu.mult)
            nc.vector.tensor_tensor(t_h1[:, :, 1], t_tmp2[:], a2[:, :, 1],
                                    Alu.add)
            nc.gpsimd.tensor_tensor(t_h2[:, :, 1], a1[:, :, 1], t_h2[:, :, 0],
                                    Alu.mult)
            # s = 2 .. L1-1
            for s in range(2, L1):
                a1s = a1[:, :, s]
                a2s = a2[:, :, s]
                nc.vector.tensor_tensor(t_tmp1[:], a1s, t_yz[:, :, s - 1],
                                        Alu.mult)
                nc.vector.tensor_tensor(t_tmp2[:], a2s, t_yz[:, :, s - 2],
                                        Alu.mult)
                nc.vector.tensor_tensor(t_tmp1[:], t_tmp1[:], t_tmp2[:],
                                        Alu.add)
                nc.vector.tensor_tensor(t_yz[:, :, s], t_x[:, :, s], t_tmp1[:],
                                        Alu.add)
                nc.vector.tensor_tensor(t_tmp1[:], a1s, t_h1[:, :, s - 1],
                                        Alu.mult)
                nc.vector.tensor_tensor(t_tmp2[:], a2s, t_h1[:, :, s - 2],
                                        Alu.mult)
                nc.vector.tensor_tensor(t_h1[:, :, s], t_tmp1[:], t_tmp2[:],
                                        Alu.add)
                nc.gpsimd.tensor_tensor(t_g1[:], a1s, t_h2[:, :, s - 1],
                                        Alu.mult)
                nc.gpsimd.tensor_tensor(t_g2[:], a2s, t_h2[:, :, s - 2],
                                        Alu.mult)
                nc.gpsimd.tensor_tensor(t_h2[:, :, s], t_g1[:], t_g2[:],
                                        Alu.add)

            # ---------------- phase C: Kogge-Stone combine ----------------
            # comps order: m11, m12, m21, m22, q1, q2
            # KS ping-pong buffers carved out of t_x (dead after the scan)
            xf = t_x[:].rearrange("p a b -> p (a b)")
            ksA = [xf[:, i * CPS:(i + 1) * CPS] for i in range(6)]
            ksB = [xf[:, (6 + i) * CPS:(7 + i) * CPS] for i in range(6)]
            srcs = [t_h1[:, :, L1 - 1], t_h2[:, :, L1 - 1],
                    t_h1[:, :, L1 - 2], t_h2[:, :, L1 - 2],
                    t_yz[:, :, L1 - 1], t_yz[:, :, L1 - 2]]
            for i in range(6):
                nc.scalar.activation(ksA[i], srcs[i], Act.Copy)

            def ks_compose(dst, hi, lo, off, w, tmps):
                """dst[c] = hi[c] o lo[c-off] for the w composable entries.
                13 ops (m11,m12,q1,m21 rows) on DVE, 7 (m22,q2) on GPSIMD."""
                (d11, d12, d21, d22, dq1, dq2) = [d[:, off:off + w] for d in dst]
                (h11, h12, h21, h22, hq1, hq2) = [h[:, off:off + w] for h in hi]
                (l11, l12, l21, l22, lq1, lq2) = [l[:, 0:w] for l in lo]
                (tA, tB, tC, tD) = tmps
                # row 1 (DVE)
                nc.vector.tensor_tensor(tA, h11, l11, Alu.mult)
                nc.vector.tensor_tensor(tB, h12, l21, Alu.mult)
                nc.vector.tensor_tensor(d11, tA, tB, Alu.add)
                nc.vector.tensor_tensor(tA, h11, l12, Alu.mult)
                nc.vector.tensor_tensor(tB, h12, l22, Alu.mult)
                nc.vector.tensor_tensor(d12, tA, tB, Alu.add)
                nc.vector.tensor_tensor(tA, h11, lq1, Alu.mult)
                nc.vector.tensor_tensor(tB, h12, lq2, Alu.mult)
                nc.vector.tensor_tensor(tA, tA, tB, Alu.add)
                nc.vector.tensor_tensor(dq1, tA, hq1, Alu.add)
                # row 2: m21 on DVE, m22/q2 on GPSIMD
                nc.vector.tensor_tensor(tA, h21, l11, Alu.mult)
                nc.vector.tensor_tensor(tB, h22, l21, Alu.mult)
                nc.vector.tensor_tensor(d21, tA, tB, Alu.add)
                nc.gpsimd.tensor_tensor(tC, h21, l12, Alu.mult)
                nc.gpsimd.tensor_tensor(tD, h22, l22, Alu.mult)
                nc.gpsimd.tensor_tensor(d22, tC, tD, Alu.add)
                nc.gpsimd.tensor_tensor(tC, h21, lq1, Alu.mult)
                nc.gpsimd.tensor_tensor(tD, h22, lq2, Alu.mult)
                nc.gpsimd.tensor_tensor(tC, tC, tD, Alu.add)
                nc.gpsimd.tensor_tensor(dq2, tC, hq2, Alu.add)

            cur, nxt = ksA, ksB
            off = 1
            while off < CPS:
                for i in range(6):  # pass-through prefix entries
                    nc.scalar.activation(nxt[i][:, 0:off], cur[i][:, 0:off],
                                         Act.Copy)
                w = CPS - off
                ks_compose(nxt, cur, cur, off, w,
                           (t_tmp1[:, 0:w], t_tmp2[:, 0:w],
                            t_g1[:, 0:w], t_g2[:, 0:w]))
                cur, nxt = nxt, cur
                off *= 2

            # stretch composites -> DRAM (st_cmp[p, c] = comp c of partition p)
            t_cmp = pool.tile([128, 6], fp32, name="cmp")
            for i in range(6):
                nc.scalar.activation(t_cmp[:, i:i + 1], cur[i][:, CPS - 1:CPS],
                                     Act.Copy)
            dma(out=st_cmp, in_=t_cmp[:])

            # row-level KS over the 32 stretches of each row (4 partitions)
            t_row = pool.tile([4, NSTR, 6], fp32, name="rowc")
            dma(out=t_row[:], in_=st_cmp.rearrange("(r j) c -> r j c", r=ROWS))
            rA = [pool.tile([4, NSTR], fp32, name=f"rA{i}") for i in range(6)]
            rB = [pool.tile([4, NSTR], fp32, name=f"rB{i}") for i in range(6)]
            rt = [pool.tile([4, NSTR], fp32, name=f"rt{i}") for i in range(4)]
            for i in range(6):
                nc.scalar.activation(rA[i][:], t_row[:, :, i], Act.Copy)
            rcur, rnxt = rA, rB
            off = 1
            while off < NSTR:
                for i in range(6):
                    nc.scalar.activation(rnxt[i][:, 0:off], rcur[i][:, 0:off],
                                         Act.Copy)
                w = NSTR - off
                ks_compose([rr[:] for rr in rnxt], [rr[:] for rr in rcur],
                           [rr[:] for rr in rcur], off, w,
                           tuple(rr[:, 0:w] for rr in (rt[0][:], rt[1][:],
                                                       rt[2][:], rt[3][:])))
                rcur, rnxt = rnxt, rcur
                off *= 2

            # stretch entry states: s_entry[j] = (q1, q2) of prefix[j-1]
            sent = pool.tile([4, NSTR, 2], fp32, name="sent")
            nc.vector.memset(sent[:, 0, :], 0.0)
            nc.vector.tensor_copy(sent[:, 1:NSTR, 0], rcur[4][:, 0:NSTR - 1])
            nc.vector.tensor_copy(sent[:, 1:NSTR, 1], rcur[5][:, 0:NSTR - 1])
            dma(out=st_sin, in_=sent[:])

            # back to scan layout: per-partition stretch entry [128, 2]
            t_sstr = pool.tile([128, 2], fp32, name="sstr")
            dma(out=t_sstr[:], in_=st_sin.rearrange("r j c -> (r j) c"))

            # chunk entry states: alpha/beta [128, CPS]
            # alpha[0] = s1 ; alpha[c] = m11p[c-1] s1 + m12p[c-1] s2 + q1p[c-1]
            s1 = t_sstr[:, 0:1]
            s2 = t_sstr[:, 1:2]
            nc.vector.tensor_copy(t_al[:, 0:1], s1)
            nc.vector.tensor_copy(t_be[:, 0:1], s2)
            W = CPS - 1
            # TensorScalarPtr (AP-scalar) ops are DVE-only; plain adds on GP
            nc.vector.tensor_scalar_mul(t_tmp1[:, 0:W], cur[0][:, 0:W], s1)
            nc.vector.scalar_tensor_tensor(t_al[:, 1:CPS], cur[1][:, 0:W], s2,
                                           t_tmp1[:, 0:W], Alu.mult, Alu.add)
            nc.gpsimd.tensor_tensor(t_al[:, 1:CPS], t_al[:, 1:CPS],
                                    cur[4][:, 0:W], Alu.add)
            nc.vector.tensor_scalar_mul(t_tmp2[:, 0:W], cur[2][:, 0:W], s1)
            nc.vector.scalar_tensor_tensor(t_be[:, 1:CPS], cur[3][:, 0:W], s2,
                                           t_tmp2[:, 0:W], Alu.mult, Alu.add)
            nc.gpsimd.tensor_tensor(t_be[:, 1:CPS], t_be[:, 1:CPS],
                                    cur[5][:, 0:W], Alu.add)

            # ---- correction (in place): y += alpha*h1 + beta*h2 ----
            alb = t_al[:].unsqueeze(-1).broadcast_to([128, CPS, L1])
            beb = t_be[:].unsqueeze(-1).broadcast_to([128, CPS, L1])
            C0 = 186  # DVE/GPSIMD column split (~73/27)
            nc.vector.tensor_tensor(t_h1[:, 0:C0], t_h1[:, 0:C0], alb[:, 0:C0],
                                    Alu.mult)
            nc.vector.tensor_tensor(t_h2[:, 0:C0], t_h2[:, 0:C0], beb[:, 0:C0],
                                    Alu.mult)
            nc.vector.tensor_tensor(t_yz[:, 0:C0], t_yz[:, 0:C0],
                                    t_h1[:, 0:C0], Alu.add)
            nc.vector.tensor_tensor(t_yz[:, 0:C0], t_yz[:, 0:C0],
                                    t_h2[:, 0:C0], Alu.add)
            nc.gpsimd.tensor_tensor(t_h1[:, C0:CPS], t_h1[:, C0:CPS],
                                    alb[:, C0:CPS], Alu.mult)
            nc.gpsimd.tensor_tensor(t_h2[:, C0:CPS], t_h2[:, C0:CPS],
                                    beb[:, C0:CPS], Alu.mult)
            nc.gpsimd.tensor_tensor(t_yz[:, C0:CPS], t_yz[:, C0:CPS],
                                    t_h1[:, C0:CPS], Alu.add)
            nc.gpsimd.tensor_tensor(t_yz[:, C0:CPS], t_yz[:, C0:CPS],
                                    t_h2[:, C0:CPS], Alu.add)

            # store corrected y to time-linear stage (with zeroed lead pad)
            zpad = pool.tile([ROWS, PAD], fp32, name="zpad")
            nc.vector.memset(zpad[:], 0.0)
            dma(out=st_y[:, 0:PAD], in_=zpad[:])
            for r in range(ROWS):
                dma(out=st_y[r, PAD:PAD + T].rearrange("(p s) -> p s", p=NSTR),
                    in_=t_yz[r * NSTR:(r + 1) * NSTR]
                    .rearrange("p a b -> p (a b)"))

        # ------------- phase D: FIR in segment-window layout -------------
        with tc.tile_pool(name="firp", bufs=2) as fp_pool, \
             tc.tile_pool(name="firy", bufs=4) as fy_pool:
            for ti in range(8):
                t_v0 = t_v0a[:, ti, :]
                t_d = t_da[:, ti, :]
                t_w0 = t_w0a[:, ti, :]
                t_w = fp_pool.tile([128, WIN], fp32, name=f"fw_{ti}", tag="fw")
                nc.scalar.activation(t_w[:], t_iota[:], Act.Identity,
                                     bias=t_w0[:, 0:1], scale=DELTA)
                t_yw = fy_pool.tile([128, WIN], fp32, name=f"yw_{ti}", tag="yw")
                for (tj, part, r, k, sp0, n) in [u for u in RUNS if u[0] == ti]:
                    start = PAD + SEGLEN * k + 1028 * sp0 - 2
                    dma(out=t_yw[part:part + n, :],
                        in_=win_src(st_y, r, start, n))
                t_b = [fp_pool.tile([128, WIN], fp32, name=f"b{j}_{ti}",
                                    tag=f"b{j}") for j in range(3)]
                for j in range(3):
                    nc.scalar.activation(t_b[j][:], t_w[:], Act.Identity,
                                         bias=t_v0[:, 2 + j:3 + j],
                                         scale=t_d[:, 2 + j:3 + j])
                t_o = fp_pool.tile([128, WIN], fp32, name=f"o_{ti}", tag="o")
                t_f1 = fp_pool.tile([128, WIN - 2], fp32, name=f"f1_{ti}",
                                    tag="f1")
                t_f2 = fp_pool.tile([128, WIN - 2], fp32, name=f"f2_{ti}",
                                    tag="f2")
                nc.vector.tensor_tensor(t_o[:, 2:], t_b[0][:, 2:], t_yw[:, 2:],
                                        Alu.mult)
                nc.vector.tensor_tensor(t_f1[:], t_b[1][:, 2:],
                                        t_yw[:, 1:WIN - 1], Alu.mult)
                nc.gpsimd.tensor_tensor(t_f2[:], t_b[2][:, 2:],
                                        t_yw[:, 0:WIN - 2], Alu.mult)
                nc.vector.tensor_tensor(t_o[:, 2:], t_o[:, 2:], t_f1[:],
                                        Alu.add)
                nc.vector.tensor_tensor(t_o[:, 2:], t_o[:, 2:], t_f2[:],
                                        Alu.add)
                scatter_tile(ti, t_o[:], y_out, 0)

    _fix_multi_waits(nc)
    return nc


_NC_CACHE = None
LAST_EXEC_NS = None


def _register_ntff_hook():
    """Make antenv.axon_hooks importable and register the ctypes NTFF hook so
    run_bass_kernel_spmd(trace=True) can measure real device exec time."""
    import types
    name = 'antenv.axon_hooks'
    if name not in sys.modules:
        mod = types.ModuleType(name)
        holder = [None]
        mod.set_axon_ntff_profile_hook = lambda h: holder.__setitem__(0, h)
        mod.get_axon_ntff_profile_hook = lambda: holder[0]
        import antenv
        antenv.axon_hooks = mod
        sys.modules[name] = mod
    if sys.modules[name].get_axon_ntff_profile_hook() is None:
        from trn_agent_boot.trn_boot import _ntff_profile_via_ctypes
        hook = _ntff_profile_via_ctypes('/opt/axon/libaxon_pjrt.so')
        sys.modules[name].set_axon_ntff_profile_hook(hook)


def kernel(x, coeff_logits):
    """Full inputs -> full output, running the Bass kernel on 8 NeuronCores."""
    global _NC_CACHE, LAST_EXEC_NS
    _patch_tile_drain()
    from concourse.bass_utils import run_bass_kernel_spmd

    x = np.ascontiguousarray(np.asarray(x, dtype=np.float32))
    cl = np.ascontiguousarray(np.asarray(coeff_logits, dtype=np.float32))
    if _NC_CACHE is None:
        _NC_CACHE = build_program()
    nc = _NC_CACHE

    w0, iota = host_tables()
    in_maps = []
    for c in range(NCORES):
        rows = slice(c * ROWS, (c + 1) * ROWS)
        v0, v1 = host_v0v1(cl[rows])
        in_maps.append({
            "x": x[rows].copy(),
            "v0": v0, "v1": v1, "w0": w0, "iota": iota,
        })
    import os
    mode = os.environ.get("KERNEL_TIME", "0")
    if mode == "ntff":
        # real device timing + perfetto trace via the NTFF profile hook
        _register_ntff_hook()
        from concourse import bass_utils as _bu
        _bu.upload_artifacts = lambda tmpdir: "local://" + tmpdir
        outdir = os.environ.get("PROF_OUT", "/tmp/kernel_prof")
        os.makedirs(outdir, exist_ok=True)
        res = run_bass_kernel_spmd(nc, in_maps, list(range(NCORES)),
                                   tmpdir=outdir, trace=True, trace_cores=[0])
        LAST_EXEC_NS = res.exec_time_ns
    elif mode == "1":
        import time, jax
        cap = {}
        orig_jit = jax.jit

        def capturing_jit(f, **kw):
            j = orig_jit(f, **kw)

            def wrapper(*a, **k):
                cap['fn'], cap['args'] = j, a
                return j(*a, **k)
            return wrapper

        jax.jit = capturing_jit
        try:
            res = run_bass_kernel_spmd(nc, in_maps, list(range(NCORES)))
        finally:
            jax.jit = orig_jit
        try:
            fn, args = cap['fn'], cap['args']
            jax.block_until_ready(fn(*args))  # warm
            K = 5
            t0 = time.perf_counter()
            for _ in range(K):
                o = fn(*args)
            jax.block_until_ready(o)
            LAST_EXEC_NS = int((time.perf_counter() - t0) / K * 1e9)
        except Exception as e:
            print("timing failed:", e)
            LAST_EXEC_NS = -1
    else:
        res = run_bass_kernel_spmd(nc, in_maps, list(range(NCORES)))
    out = np.empty((B, T), np.float32)
    for c in range(NCORES):
        out[c * ROWS:(c + 1) * ROWS] = res.results[c]["y"]
    return out


# revision 16
# speedup vs baseline: 1.2203x; 1.0282x over previous
"""Trainium2 Bass kernel for nn_BiquadCoeffFilter_31628139167986.

Reference computation (per batch row, T = 262144 samples):
  logits = linear-interp of 256 control points -> T samples (5 channels)
  a1 = 2*tanh(l0)*stab ; a2 = 0.5*((2-|a1|)*tanh(l1)*stab + |a1|)  (stab = 1-1e-3)
  IIR:  y[t] = x[t] - a1[t]*y[t-1] - a2[t]*y[t-2]
  FIR:  out[t] = b0[t]*y[t] + b1[t]*y[t-1] + b2[t]*y[t-2],  b = logits[..., 2:5]

Sharding: pure data parallel, 4 batch rows per core x 8 NeuronCores (SPMD).

Per-core pipeline (v3):
  A. a-coefficient generation in SEGMENT-WINDOW layout (partition = one
     interpolation segment window of 1032 samples; the interpolated logit is
     affine in the in-window position, so the Activation engine computes
     tanh(w*d + v0) with per-partition scale/bias).  na1/na2 scattered to
     per-row time-linear DRAM stages (issues split SP/GPSIMD), reloaded per
     row into the scan tile.
  B. Chunked 3-solution scan in SCAN layout [128 partitions = 8192-sample
     stretches, 256 chunks x 32 steps]: zero-state response y_zero +
     homogeneous h1 on DVE, h2 on GPSIMD.
  C. Kogge-Stone prefix over the 256 per-chunk 2x2 affine transfer maps
     (flat [128,256] comps; 13 ops on DVE / 7 on GPSIMD per level), a [4,32]
     stretch-level KS via a tiny DRAM hop, per-chunk entry states
     alpha/beta, then the in-place correction y += alpha*h1 + beta*h2.
  D. FIR in segment-window layout: corrected y staged to DRAM per row,
     windows gathered back (deep prefetch), b coefficients generated on the
     Activation engine, output scattered straight to DRAM (issues split
     SP/GPSIMD).
"""
import sys
sys.path.insert(0, '/opt/trn_rl_repo')
import numpy as np

B, T = 32, 262144
NSEG = 255
SEGLEN = 87381      # (T-1)/3 ; 3 super-blocks x 85 segments per row
SUP = 85
ROWS = 4
NCORES = 8
L1 = 32             # chunk length
NSTR = 32           # stretches per row
STR = T // NSTR     # 8192
CPS = STR // L1     # 256 chunks per stretch
WIN = 1032
PAD = 4
DELTA = float(NSEG) / float(T - 1)
STAB = 1.0 - 1e-3

_PATCHED = False


def _patch_tile_drain():
    """This toolchain allows a single sem wait per instruction; split the tile
    tail-drain's accumulated waits across chained drain instructions."""
    global _PATCHED
    if _PATCHED:
        return
    from concourse import tile, mybir
    from concourse.vector_clock import ScopedClock

    def _drain_and_barrier_split(self, tick_clock, wait_clock):
        drain_inst = self.nc.sync.drain()
        wait_clock.add_sem_waits(
            drain_inst.ins, ScopedClock({None: tick_clock.global_clock}))
        si = drain_inst.ins.sync_info
        waits = list(si.on_wait or []) if si else []
        if len(waits) > 1:
            si.on_wait = waits[:1]
            for i in range(1, len(waits)):
                d2 = self.nc.sync.drain()
                d2.ins.sync_info = mybir.SyncInfo(on_wait=[waits[i]], on_update=[])
        self.nc.all_engine_barrier()
        assert self.sems is not None
        popped = self.nc._tile_sem_poison_stack.pop()
        assert popped is self._sem_poison
        self.nc.clear_and_free_semaphores(list(self.sems.allocated().values()))
        self.nc.all_engine_barrier()

    tile.TileContext._drain_and_barrier = _drain_and_barrier_split
    _PATCHED = True


def _fix_multi_waits(nc):
    """Hoist extra sem waits onto same-engine nops (1-wait codegen limit)."""
    from concourse import mybir

    def make_nop(engine):
        bi = nc.engines[engine].nop(nofuse=True, hint="wait_split")
        inst = bi.ins
        for f in nc.m.functions:
            for bb in f.blocks:
                il = bb.instructions
                if il and il[-1] is inst:
                    bb.instructions = il[:-1]
                    return inst
        raise RuntimeError("nop not found")

    for f in nc.m.functions:
        for bb in f.blocks:
            il = list(bb.instructions)
            out = []
            changed = False
            for inst in il:
                si = getattr(inst, 'sync_info', None)
                waits = list(si.on_wait or []) if si else []
                if len(waits) > 1 and getattr(inst, 'engine', None) is not None:
                    changed = True
                    extra, keep = waits[:-1], waits[-1:]
                    for w in extra:
                        nop = make_nop(inst.engine)
                        nop.sync_info = mybir.SyncInfo(on_wait=[w], on_update=[])
                        out.append(nop)
                    si.on_wait = keep
                out.append(inst)
            if changed:
                bb.instructions = out
    return nc


def _lane_runs():
    """lane = r*255 + 85*k + sp  (row r, super-block k, segment sp).
    Runs of consecutive sp split at 128-partition tile boundaries.
    Returns (tile, part0, r, k, sp0, n)."""
    runs = []
    for r in range(ROWS):
        for k in range(3):
            base = r * NSEG + SUP * k
            sp = 0
            while sp < SUP:
                lane = base + sp
                tile_i, part = divmod(lane, 128)
                n = min(SUP - sp, 128 - part)
                runs.append((tile_i, part, r, k, sp, n))
                sp += n
    return runs


RUNS = _lane_runs()


def host_tables():
    w0 = np.zeros((128, 8, 1), np.float32)
    for r in range(ROWS):
        for k in range(3):
            for sp in range(SUP):
                lane = r * NSEG + SUP * k + sp
                seg = SUP * k + sp
                wstart = SEGLEN * k + 1028 * sp - 2
                w0[lane % 128, lane // 128, 0] = np.float64(wstart) * DELTA - seg
    iota = np.arange(WIN, dtype=np.float32)[None, :].repeat(128, 0)
    return w0, iota


def host_v0v1(cl_rows):
    """Per-lane control-point values [8,128,5] (pure data movement)."""
    v0 = np.zeros((128, 8, 5), np.float32)
    v1 = np.zeros((128, 8, 5), np.float32)
    for r in range(ROWS):
        for seg in range(NSEG):
            lane = r * NSEG + seg
            v0[lane % 128, lane // 128] = cl_rows[r, seg]
            v1[lane % 128, lane // 128] = cl_rows[r, seg + 1]
    return v0, v1


def build_program():
    from concourse import bass, mybir
    from concourse.tile import TileContext
    import bass_rust
    fp32 = mybir.dt.float32
    Alu = mybir.AluOpType
    Act = mybir.ActivationFunctionType

    nc = bass.Bass("TRN2", target_bir_lowering=False, debug=False)

    x_in = nc.dram_tensor("x", [ROWS, T], fp32, kind="ExternalInput").ap()
    v0_in = nc.dram_tensor("v0", [128, 8, 5], fp32, kind="ExternalInput").ap()
    v1_in = nc.dram_tensor("v1", [128, 8, 5], fp32, kind="ExternalInput").ap()
    w0_in = nc.dram_tensor("w0", [128, 8, 1], fp32, kind="ExternalInput").ap()
    iota_in = nc.dram_tensor("iota", [128, WIN], fp32, kind="ExternalInput").ap()
    y_out = nc.dram_tensor("y", [ROWS, T], fp32, kind="ExternalOutput").ap()

    st_nas = [nc.dram_tensor(f"st_na{r}", [2, T + 2 * PAD], fp32).ap()
              for r in range(ROWS)]
    st_y = nc.dram_tensor("st_y", [ROWS, T + 2 * PAD], fp32).ap()
    st_cmp = nc.dram_tensor("st_cmp", [128, 6], fp32).ap()
    st_sin = nc.dram_tensor("st_sin", [ROWS, NSTR, 2], fp32).ap()

    # Scatter-heavy DMA issuance alternates SP / GPSIMD (both near idle at
    # scatter time); everything latency-critical goes on SP.
    _sc_engines = [nc.sync, nc.gpsimd]
    _sc_rr = [0]

    def dma(out, in_, eng=None):
        return (eng or nc.sync).dma_start(out=out, in_=in_)

    def dma_rr(out, in_):
        eng = _sc_engines[_sc_rr[0] % len(_sc_engines)]
        _sc_rr[0] += 1
        return eng.dma_start(out=out, in_=in_)

    def win_src(st, r, start, n):
        v = st[r, :].copy()
        v.ap = bass_rust.VecI64Pair([[1028, n], [1, WIN]])
        v.offset = v.offset + start
        return v

    def na_dst(r, start, n, ln):
        # [n segs (stride 1028), 2 planes, ln cols] view into row-r na stage
        v = st_nas[r][0, :].copy()
        v.ap = bass_rust.VecI64Pair([[1028, n], [T + 2 * PAD, 2], [1, ln]])
        v.offset = v.offset + start
        return v

    def scatter_na(ti, tap):
        """Scatter BOTH na planes of the paired tile [128, 2*WIN] at once."""
        tv = tap.rearrange("p (c w) -> p c w", c=2)
        for (tj, part, r, k, sp0, n) in [u for u in RUNS if u[0] == ti]:
            if sp0 == 0:
                dma_rr(out=na_dst(r, PAD + SEGLEN * k, 1, 1029),
                       in_=tv[part:part + 1, :, 2:WIN - 1])
                if n > 1:
                    dma_rr(out=na_dst(r, PAD + SEGLEN * k + 1029, n - 1, 1028),
                           in_=tv[part + 1: part + n, :, 3:WIN - 1])
            else:
                dma_rr(out=na_dst(r, PAD + SEGLEN * k + 1028 * sp0 + 1, n, 1028),
                       in_=tv[part: part + n, :, 3:WIN - 1])
        for r in range(ROWS):
            lane = r * NSEG + NSEG - 1
            tj2, part2 = divmod(lane, 128)
            if tj2 == ti:
                dma_rr(out=na_dst(r, PAD + T - 1, 1, 1),
                       in_=tv[part2:part2 + 1, :, WIN - 1:WIN])

    def scatter_tile(ti, tap, st, off):
        """Write true-segment cols of window tile `ti` to time-linear layout
        (issues on SP only — GPSIMD owns the phase-D gathers)."""
        for (tj, part, r, k, sp0, n) in [u for u in RUNS if u[0] == ti]:
            if sp0 == 0:
                base_t = SEGLEN * k
                dma(out=st[r:r + 1, off + base_t: off + base_t + 1029],
                    in_=tap[part:part + 1, 2:WIN - 1])
                if n > 1:
                    base_t = SEGLEN * k + 1029
                    dma(
                        out=st[r, off + base_t: off + base_t + (n - 1) * 1028]
                            .rearrange("(a b) -> a b", b=1028),
                        in_=tap[part + 1: part + n, 3:WIN - 1])
            else:
                base_t = SEGLEN * k + 1028 * sp0 + 1
                dma(
                    out=st[r, off + base_t: off + base_t + n * 1028]
                        .rearrange("(a b) -> a b", b=1028),
                    in_=tap[part: part + n, 3:WIN - 1])
        # last sample t = T-1 comes from the final segment's window col 1031
        for r in range(ROWS):
            lane = r * NSEG + NSEG - 1
            tj2, part2 = divmod(lane, 128)
            if tj2 == ti:
                dma(out=st[r:r + 1, off + T - 1: off + T],
                    in_=tap[part2:part2 + 1, WIN - 1:WIN])

    with TileContext(nc) as tc:
      with tc.tile_pool(name="outer", bufs=1) as outer, \
           tc.tile_pool(name="consts", bufs=1) as consts:
        t_x = outer.tile([128, CPS, L1], fp32, name="xs")
        t_a12 = outer.tile([128, 2, CPS, L1], fp32, name="a12")
        t_yz = outer.tile([128, CPS, L1], fp32, name="yzs")
        dma(out=t_x[:].rearrange("p a b -> p (a b)"),
            in_=x_in.rearrange("r (p s) -> (r p) s", p=NSTR))

        t_iota = consts.tile([128, WIN], fp32, name="iota_t")
        t_v0a = consts.tile([128, 8, 5], fp32, name="v0all")
        t_v1a = consts.tile([128, 8, 5], fp32, name="v1all")
        t_w0a = consts.tile([128, 8, 1], fp32, name="w0all")
        t_da = consts.tile([128, 8, 5], fp32, name="dall")
        dma(out=t_iota[:], in_=iota_in)
        dma(out=t_v0a[:], in_=v0_in)
        dma(out=t_v1a[:], in_=v1_in)
        dma(out=t_w0a[:], in_=w0_in)
        nc.vector.tensor_tensor(t_da[:], t_v1a[:], t_v0a[:], Alu.subtract)

        # ---------------- phase A: a-coefficient generation ----------------
        with tc.tile_pool(name="segp", bufs=2) as sp_pool:
            for ti in range(8):
                t_v0 = t_v0a[:, ti, :]
                t_d = t_da[:, ti, :]
                t_w0 = t_w0a[:, ti, :]

                t_w = sp_pool.tile([128, WIN], fp32, name=f"w_{ti}", tag="w")
                nc.scalar.activation(t_w[:], t_iota[:], Act.Identity,
                                     bias=t_w0[:, 0:1], scale=DELTA)
                t_t1 = sp_pool.tile([128, WIN], fp32, name=f"t1_{ti}", tag="t1")
                t_t2 = sp_pool.tile([128, WIN], fp32, name=f"t2_{ti}", tag="t2")
                nc.scalar.activation(t_t1[:], t_w[:], Act.Tanh,
                                     bias=t_v0[:, 0:1], scale=t_d[:, 0:1])
                nc.scalar.activation(t_t2[:], t_w[:], Act.Tanh,
                                     bias=t_v0[:, 1:2], scale=t_d[:, 1:2])
                t_u = sp_pool.tile([128, WIN], fp32, name=f"u_{ti}", tag="u")
                nc.scalar.activation(t_u[:], t_t1[:], Act.Abs)
                t_na = sp_pool.tile([128, 2 * WIN], fp32, name=f"na_{ti}",
                                    tag="na")
                t_na1 = t_na[:, 0:WIN]
                t_na2 = t_na[:, WIN:2 * WIN]
                nc.vector.tensor_scalar_mul(t_na1, t_t1[:], -2.0 * STAB)
                t_st2 = sp_pool.tile([128, WIN], fp32, name=f"st2_{ti}",
                                     tag="st2")
                nc.vector.tensor_scalar_mul(t_st2[:], t_t2[:], STAB)
                t_vv = sp_pool.tile([128, WIN], fp32, name=f"vv_{ti}", tag="vv")
                nc.vector.tensor_scalar(t_vv[:], t_st2[:], -1.0, 1.0,
                                        Alu.mult, Alu.add)
                t_uv = sp_pool.tile([128, WIN], fp32, name=f"uv_{ti}", tag="uv")
                nc.gpsimd.tensor_tensor(t_uv[:], t_u[:], t_vv[:], Alu.mult)
                nc.vector.scalar_tensor_tensor(t_na2, t_uv[:], -STAB, t_st2[:],
                                               Alu.mult, Alu.subtract)
                scatter_na(ti, t_na[:])

        # per-row a12 reloads: RAW-dep only on that row's scatters
        a1 = t_a12[:, 0, :, :]
        a2 = t_a12[:, 1, :, :]
        for r in range(ROWS):
            dma(out=t_a12[r * NSTR:(r + 1) * NSTR, 0]
                .rearrange("p a b -> p (a b)"),
                in_=st_nas[r][0, PAD:PAD + T].rearrange("(p s) -> p s", p=NSTR))
            dma(out=t_a12[r * NSTR:(r + 1) * NSTR, 1]
                .rearrange("p a b -> p (a b)"),
                in_=st_nas[r][1, PAD:PAD + T].rearrange("(p s) -> p s", p=NSTR))

        # ---------------- phase B: chunked 3-solution scan ----------------
        with tc.tile_pool(name="scanp", bufs=1) as pool:
            t_h1 = pool.tile([128, CPS, L1], fp32, name="h1s")
            t_h2 = pool.tile([128, CPS, L1], fp32, name="h2s")
            t_tmp1 = pool.tile([128, CPS], fp32, name="sc_t1")
            t_tmp2 = pool.tile([128, CPS], fp32, name="sc_t2")
            t_g1 = pool.tile([128, CPS], fp32, name="gp_t1")
            t_g2 = pool.tile([128, CPS], fp32, name="gp_t2")
            t_al = pool.tile([128, CPS], fp32, name="alph")
            t_be = pool.tile([128, CPS], fp32, name="beta")

            # s = 0:  yz0 = x0 ; h1 col0 = a1_0 ; h2 col0 = a2_0
            nc.scalar.activation(t_yz[:, :, 0], t_x[:, :, 0], Act.Copy)
            nc.scalar.activation(t_h1[:, :, 0], a1[:, :, 0], Act.Copy)
            nc.scalar.activation(t_h2[:, :, 0], a2[:, :, 0], Act.Copy)
            # s = 1
            nc.vector.tensor_tensor(t_tmp1[:], a1[:, :, 1], t_yz[:, :, 0],
                                    Alu.mult)
            nc.vector.tensor_tensor(t_yz[:, :, 1], t_x[:, :, 1], t_tmp1[:],
                                    Alu.add)
            nc.vector.tensor_tensor(t_tmp2[:], a1[:, :, 1], t_h1[:, :, 0],
                                    Alu.mult)
            nc.vector.tensor_tensor(t_h1[:, :, 1], t_tmp2[:], a2[:, :, 1],
                                    Alu.add)
            nc.gpsimd.tensor_tensor(t_h2[:, :, 1], a1[:, :, 1], t_h2[:, :, 0],
                                    Alu.mult)
            # s = 2 .. L1-1
            for s in range(2, L1):
                a1s = a1[:, :, s]
                a2s = a2[:, :, s]
                nc.vector.tensor_tensor(t_tmp1[:], a1s, t_yz[:, :, s - 1],
                                        Alu.mult)
                nc.vector.tensor_tensor(t_tmp2[:], a2s, t_yz[:, :, s - 2],
                                        Alu.mult)
                nc.vector.tensor_tensor(t_tmp1[:], t_tmp1[:], t_tmp2[:],
                                        Alu.add)
                nc.vector.tensor_tensor(t_yz[:, :, s], t_x[:, :, s], t_tmp1[:],
                                        Alu.add)
                nc.vector.tensor_tensor(t_tmp1[:], a1s, t_h1[:, :, s - 1],
                                        Alu.mult)
                nc.vector.tensor_tensor(t_tmp2[:], a2s, t_h1[:, :, s - 2],
                                        Alu.mult)
                nc.vector.tensor_tensor(t_h1[:, :, s], t_tmp1[:], t_tmp2[:],
                                        Alu.add)
                nc.gpsimd.tensor_tensor(t_g1[:], a1s, t_h2[:, :, s - 1],
                                        Alu.mult)
                nc.gpsimd.tensor_tensor(t_g2[:], a2s, t_h2[:, :, s - 2],
                                        Alu.mult)
                nc.gpsimd.tensor_tensor(t_h2[:, :, s], t_g1[:], t_g2[:],
                                        Alu.add)

            # ---------------- phase C: Kogge-Stone combine ----------------
            # comps order: m11, m12, m21, m22, q1, q2
            # KS ping-pong buffers carved out of t_x (dead after the scan)
            xf = t_x[:].rearrange("p a b -> p (a b)")
            ksA = [xf[:, i * CPS:(i + 1) * CPS] for i in range(6)]
            ksB = [xf[:, (6 + i) * CPS:(7 + i) * CPS] for i in range(6)]
            srcs = [t_h1[:, :, L1 - 1], t_h2[:, :, L1 - 1],
                    t_h1[:, :, L1 - 2], t_h2[:, :, L1 - 2],
                    t_yz[:, :, L1 - 1], t_yz[:, :, L1 - 2]]
            for i in range(6):
                nc.scalar.activation(ksA[i], srcs[i], Act.Copy)

            def ks_compose(dst, hi, lo, off, w, tmps):
                """dst[c] = hi[c] o lo[c-off] for the w composable entries.
                13 ops (m11,m12,q1,m21 rows) on DVE, 7 (m22,q2) on GPSIMD."""
                (d11, d12, d21, d22, dq1, dq2) = [d[:, off:off + w] for d in dst]
                (h11, h12, h21, h22, hq1, hq2) = [h[:, off:off + w] for h in hi]
                (l11, l12, l21, l22, lq1, lq2) = [l[:, 0:w] for l in lo]
                (tA, tB, tC, tD) = tmps
                # row 1 (DVE)
                nc.vector.tensor_tensor(tA, h11, l11, Alu.mult)
                nc.vector.tensor_tensor(tB, h12, l21, Alu.mult)
                nc.vector.tensor_tensor(d11, tA, tB, Alu.add)
                nc.vector.tensor_tensor(tA, h11, l12, Alu.mult)
                nc.vector.tensor_tensor(tB, h12, l22, Alu.mult)
                nc.vector.tensor_tensor(d12, tA, tB, Alu.add)
                nc.vector.tensor_tensor(tA, h11, lq1, Alu.mult)
                nc.vector.tensor_tensor(tB, h12, lq2, Alu.mult)
                nc.vector.tensor_tensor(tA, tA, tB, Alu.add)
                nc.vector.tensor_tensor(dq1, tA, hq1, Alu.add)
                # row 2: m21 on DVE, m22/q2 on GPSIMD
                nc.vector.tensor_tensor(tA, h21, l11, Alu.mult)
                nc.vector.tensor_tensor(tB, h22, l21, Alu.mult)
                nc.vector.tensor_tensor(d21, tA, tB, Alu.add)
                nc.gpsimd.tensor_tensor(tC, h21, l12, Alu.mult)
                nc.gpsimd.tensor_tensor(tD, h22, l22, Alu.mult)
                nc.gpsimd.tensor_tensor(d22, tC, tD, Alu.add)
                nc.gpsimd.tensor_tensor(tC, h21, lq1, Alu.mult)
                nc.gpsimd.tensor_tensor(tD, h22, lq2, Alu.mult)
                nc.gpsimd.tensor_tensor(tC, tC, tD, Alu.add)
                nc.gpsimd.tensor_tensor(dq2, tC, hq2, Alu.add)

            cur, nxt = ksA, ksB
            off = 1
            while off < CPS:
                for i in range(6):  # pass-through prefix entries
                    nc.scalar.activation(nxt[i][:, 0:off], cur[i][:, 0:off],
                                         Act.Copy)
                w = CPS - off
                ks_compose(nxt, cur, cur, off, w,
                           (t_tmp1[:, 0:w], t_tmp2[:, 0:w],
                            t_g1[:, 0:w], t_g2[:, 0:w]))
                cur, nxt = nxt, cur
                off *= 2

            # stretch composites -> [4, 32, 6] row layout via one SBUF->SBUF
            # DMA (stream order (r j) c matches on both sides)
            t_cmp = pool.tile([128, 6], fp32, name="cmp")
            for i in range(6):
                nc.scalar.activation(t_cmp[:, i:i + 1], cur[i][:, CPS - 1:CPS],
                                     Act.Copy)
            t_row = pool.tile([4, NSTR, 6], fp32, name="rowc")
            dma(out=t_row[:].rearrange("p a b -> p (a b)"), in_=t_cmp[:])

            # row-level KS over the 32 stretches of each row, entirely on DVE
            # (cross-engine sem latency dominates these tiny ops)
            rA = [pool.tile([4, NSTR], fp32, name=f"rA{i}") for i in range(6)]
            rB = [pool.tile([4, NSTR], fp32, name=f"rB{i}") for i in range(6)]
            rt = [pool.tile([4, NSTR], fp32, name=f"rt{i}") for i in range(2)]
            for i in range(6):
                nc.vector.tensor_copy(rA[i][:], t_row[:, :, i])

            def ks_compose_dve(dst, hi, lo, off, w, tA, tB):
                dd = [d[:, off:off + w] for d in dst]
                hh = [h[:, off:off + w] for h in hi]
                ll = [l[:, 0:w] for l in lo]
                for (di, hi1, hi2, lo1, lo2, acc) in (
                        (0, 0, 1, 0, 2, None), (1, 0, 1, 1, 3, None),
                        (4, 0, 1, 4, 5, 4),
                        (2, 2, 3, 0, 2, None), (3, 2, 3, 1, 3, None),
                        (5, 2, 3, 4, 5, 5)):
                    nc.vector.tensor_tensor(tA, hh[hi1], ll[lo1], Alu.mult)
                    nc.vector.tensor_tensor(tB, hh[hi2], ll[lo2], Alu.mult)
                    nc.vector.tensor_tensor(dd[di], tA, tB, Alu.add)
                    if acc is not None:
                        nc.vector.tensor_tensor(dd[di], dd[di], hh[acc],
                                                Alu.add)
                for i in range(6):
                    nc.vector.tensor_copy(dst[i][:, 0:off], lo[i][:, 0:off])

            rcur, rnxt = rA, rB
            off = 1
            while off < NSTR:
                w = NSTR - off
                ks_compose_dve([rr[:] for rr in rnxt], [rr[:] for rr in rcur],
                               [rr[:] for rr in rcur], off, w,
                               rt[0][:, 0:w], rt[1][:, 0:w])
                rcur, rnxt = rnxt, rcur
                off *= 2

            # stretch entry states: s_entry[j] = (q1, q2) of prefix[j-1];
            # back to scan layout [128, 2] via one SBUF->SBUF DMA
            sent = pool.tile([4, NSTR, 2], fp32, name="sent")
            nc.vector.memset(sent[:, 0, :], 0.0)
            nc.vector.tensor_copy(sent[:, 1:NSTR, 0], rcur[4][:, 0:NSTR - 1])
            nc.vector.tensor_copy(sent[:, 1:NSTR, 1], rcur[5][:, 0:NSTR - 1])
            t_sstr = pool.tile([128, 2], fp32, name="sstr")
            dma(out=t_sstr[:], in_=sent[:].rearrange("p a b -> p (a b)"))

            # chunk entry states: alpha/beta [128, CPS]
            # alpha[0] = s1 ; alpha[c] = m11p[c-1] s1 + m12p[c-1] s2 + q1p[c-1]
            s1 = t_sstr[:, 0:1]
            s2 = t_sstr[:, 1:2]
            nc.vector.tensor_copy(t_al[:, 0:1], s1)
            nc.vector.tensor_copy(t_be[:, 0:1], s2)
            W = CPS - 1
            # TensorScalarPtr (AP-scalar) ops are DVE-only; plain adds on GP
            nc.vector.tensor_scalar_mul(t_tmp1[:, 0:W], cur[0][:, 0:W], s1)
            nc.vector.scalar_tensor_tensor(t_al[:, 1:CPS], cur[1][:, 0:W], s2,
                                           t_tmp1[:, 0:W], Alu.mult, Alu.add)
            nc.vector.tensor_tensor(t_al[:, 1:CPS], t_al[:, 1:CPS],
                                    cur[4][:, 0:W], Alu.add)
            nc.vector.tensor_scalar_mul(t_tmp2[:, 0:W], cur[2][:, 0:W], s1)
            nc.vector.scalar_tensor_tensor(t_be[:, 1:CPS], cur[3][:, 0:W], s2,
                                           t_tmp2[:, 0:W], Alu.mult, Alu.add)
            nc.vector.tensor_tensor(t_be[:, 1:CPS], t_be[:, 1:CPS],
                                    cur[5][:, 0:W], Alu.add)

            # ---- correction (in place): y += alpha*h1 + beta*h2 ----
            alb = t_al[:].unsqueeze(-1).broadcast_to([128, CPS, L1])
            beb = t_be[:].unsqueeze(-1).broadcast_to([128, CPS, L1])
            C0 = 186  # DVE/GPSIMD column split (~73/27)
            nc.vector.tensor_tensor(t_h1[:, 0:C0], t_h1[:, 0:C0], alb[:, 0:C0],
                                    Alu.mult)
            nc.vector.tensor_tensor(t_h2[:, 0:C0], t_h2[:, 0:C0], beb[:, 0:C0],
                                    Alu.mult)
            nc.vector.tensor_tensor(t_yz[:, 0:C0], t_yz[:, 0:C0],
                                    t_h1[:, 0:C0], Alu.add)
            nc.vector.tensor_tensor(t_yz[:, 0:C0], t_yz[:, 0:C0],
                                    t_h2[:, 0:C0], Alu.add)
            nc.gpsimd.tensor_tensor(t_h1[:, C0:CPS], t_h1[:, C0:CPS],
                                    alb[:, C0:CPS], Alu.mult)
            nc.gpsimd.tensor_tensor(t_h2[:, C0:CPS], t_h2[:, C0:CPS],
                                    beb[:, C0:CPS], Alu.mult)
            nc.gpsimd.tensor_tensor(t_yz[:, C0:CPS], t_yz[:, C0:CPS],
                                    t_h1[:, C0:CPS], Alu.add)
            nc.gpsimd.tensor_tensor(t_yz[:, C0:CPS], t_yz[:, C0:CPS],
                                    t_h2[:, C0:CPS], Alu.add)

            # store corrected y to time-linear stage (with zeroed lead pad)
            zpad = pool.tile([ROWS, PAD], fp32, name="zpad")
            nc.vector.memset(zpad[:], 0.0)
            dma(out=st_y[:, 0:PAD], in_=zpad[:])
            for r in range(ROWS):
                dma(out=st_y[r, PAD:PAD + T].rearrange("(p s) -> p s", p=NSTR),
                    in_=t_yz[r * NSTR:(r + 1) * NSTR]
                    .rearrange("p a b -> p (a b)"))

        # ------------- phase D: FIR in segment-window layout -------------
        # y-window gathers issue on GPSIMD, output scatters on SP (see
        # scatter_tile's dma_rr: reset the RR so scatters start on SP and the
        # per-tile gather storm stays on GPSIMD)
        with tc.tile_pool(name="firp", bufs=2) as fp_pool, \
             tc.tile_pool(name="firo", bufs=2) as fo_pool, \
             tc.tile_pool(name="firy", bufs=3) as fy_pool:
            for ti in range(8):
                t_v0 = t_v0a[:, ti, :]
                t_d = t_da[:, ti, :]
                t_w0 = t_w0a[:, ti, :]
                t_w = fp_pool.tile([128, WIN], fp32, name=f"fw_{ti}", tag="fw")
                nc.scalar.activation(t_w[:], t_iota[:], Act.Identity,
                                     bias=t_w0[:, 0:1], scale=DELTA)
                t_yw = fy_pool.tile([128, WIN], fp32, name=f"yw_{ti}", tag="yw")
                for (tj, part, r, k, sp0, n) in [u for u in RUNS if u[0] == ti]:
                    start = PAD + SEGLEN * k + 1028 * sp0 - 2
                    dma(out=t_yw[part:part + n, :],
                        in_=win_src(st_y, r, start, n), eng=nc.gpsimd)
                t_b = [fp_pool.tile([128, WIN], fp32, name=f"b{j}_{ti}",
                                    tag=f"b{j}") for j in range(3)]
                for j in range(3):
                    nc.scalar.activation(t_b[j][:], t_w[:], Act.Identity,
                                         bias=t_v0[:, 2 + j:3 + j],
                                         scale=t_d[:, 2 + j:3 + j])
                t_o = fo_pool.tile([128, WIN], fp32, name=f"o_{ti}", tag="o")
                t_f1 = fp_pool.tile([128, WIN - 2], fp32, name=f"f1_{ti}",
                                    tag="f1")
                t_f2 = fp_pool.tile([128, WIN - 2], fp32, name=f"f2_{ti}",
                                    tag="f2")
                nc.vector.tensor_tensor(t_o[:, 2:], t_b[0][:, 2:], t_yw[:, 2:],
                                        Alu.mult)
                nc.vector.tensor_tensor(t_f1[:], t_b[1][:, 2:],
                                        t_yw[:, 1:WIN - 1], Alu.mult)
                nc.gpsimd.tensor_tensor(t_f2[:], t_b[2][:, 2:],
                                        t_yw[:, 0:WIN - 2], Alu.mult)
                nc.vector.tensor_tensor(t_o[:, 2:], t_o[:, 2:], t_f1[:],
                                        Alu.add)
                nc.vector.tensor_tensor(t_o[:, 2:], t_o[:, 2:], t_f2[:],
                                        Alu.add)
                scatter_tile(ti, t_o[:], y_out, 0)

    _fix_multi_waits(nc)
    return nc


_NC_CACHE = None
LAST_EXEC_NS = None


def _register_ntff_hook():
    """Make antenv.axon_hooks importable and register the ctypes NTFF hook so
    run_bass_kernel_spmd(trace=True) can measure real device exec time."""
    import types
    name = 'antenv.axon_hooks'
    if name not in sys.modules:
        mod = types.ModuleType(name)
        holder = [None]
        mod.set_axon_ntff_profile_hook = lambda h: holder.__setitem__(0, h)
        mod.get_axon_ntff_profile_hook = lambda: holder[0]
        import antenv
        antenv.axon_hooks = mod
        sys.modules[name] = mod
    if sys.modules[name].get_axon_ntff_profile_hook() is None:
        from trn_agent_boot.trn_boot import _ntff_profile_via_ctypes
        hook = _ntff_profile_via_ctypes('/opt/axon/libaxon_pjrt.so')
        sys.modules[name].set_axon_ntff_profile_hook(hook)


def kernel(x, coeff_logits):
    """Full inputs -> full output, running the Bass kernel on 8 NeuronCores."""
    global _NC_CACHE, LAST_EXEC_NS
    _patch_tile_drain()
    from concourse.bass_utils import run_bass_kernel_spmd

    x = np.ascontiguousarray(np.asarray(x, dtype=np.float32))
    cl = np.ascontiguousarray(np.asarray(coeff_logits, dtype=np.float32))
    if _NC_CACHE is None:
        _NC_CACHE = build_program()
    nc = _NC_CACHE

    w0, iota = host_tables()
    in_maps = []
    for c in range(NCORES):
        rows = slice(c * ROWS, (c + 1) * ROWS)
        v0, v1 = host_v0v1(cl[rows])
        in_maps.append({
            "x": x[rows].copy(),
            "v0": v0, "v1": v1, "w0": w0, "iota": iota,
        })
    import os
    mode = os.environ.get("KERNEL_TIME", "0")
    if mode == "ntff":
        # real device timing + perfetto trace via the NTFF profile hook
        _register_ntff_hook()
        from concourse import bass_utils as _bu
        _bu.upload_artifacts = lambda tmpdir: "local://" + tmpdir
        outdir = os.environ.get("PROF_OUT", "/tmp/kernel_prof")
        os.makedirs(outdir, exist_ok=True)
        res = run_bass_kernel_spmd(nc, in_maps, list(range(NCORES)),
                                   tmpdir=outdir, trace=True, trace_cores=[0])
        LAST_EXEC_NS = res.exec_time_ns
    elif mode == "1":
        import time, jax
        cap = {}
        orig_jit = jax.jit

        def capturing_jit(f, **kw):
            j = orig_jit(f, **kw)

            def wrapper(*a, **k):
                cap['fn'], cap['args'] = j, a
                return j(*a, **k)
            return wrapper

        jax.jit = capturing_jit
        try:
            res = run_bass_kernel_spmd(nc, in_maps, list(range(NCORES)))
        finally:
            jax.jit = orig_jit
        try:
            fn, args = cap['fn'], cap['args']
            jax.block_until_ready(fn(*args))  # warm
            K = 5
            t0 = time.perf_counter()
            for _ in range(K):
                o = fn(*args)
            jax.block_until_ready(o)
            LAST_EXEC_NS = int((time.perf_counter() - t0) / K * 1e9)
        except Exception as e:
            print("timing failed:", e)
            LAST_EXEC_NS = -1
    else:
        res = run_bass_kernel_spmd(nc, in_maps, list(range(NCORES)))
    out = np.empty((B, T), np.float32)
    for c in range(NCORES):
        out[c * ROWS:(c + 1) * ROWS] = res.results[c]["y"]
    return out
